# revision 50
# baseline (speedup 1.0000x reference)
"""Trainium2 Bass kernel for nn_MultiHeadAttention_46093589021200.

Causal MHA: B=4, S=2048, E=1024, H=16, D=64, with the reference's
"no-transpose-back" reshape (b,h,s,d)->(b,s,e) before the output projection.

Sharding: pure head-parallel, 2 heads per core, zero collectives.
Because of the reshape quirk, output rows s' in [h*128,(h+1)*128) depend only
on head h, so each core produces two independent 128-row output bands per
batch.

v2 design notes (vs the v1 baseline; 344250ns -> 241624ns):
  - q/k projected via PE into qkT [d2, s] head-major; v projected separately
    in NATURAL [s, d] layout (lhsT = x^T chunk), which is exactly the PV lhsT
    layout -> no DMA xbar transposes at all.
  - v bias folded into an effective o_proj bias on host (softmax rows sum to
    1, so + bv commutes through the attention average); o_proj bias added by
    DVE during the PSUM->SBUF copy (no K=1 bias matmuls).
  - attention in 512-wide q bands: scoresT [k,q] per 128-k chunk, the two
    heads in the two separate banks of one [128,1024] PSUM tile; one exp ACT
    instr per chunk covering both heads; triangular fp16 mask multiply on
    diagonal chunks; PV with v_aug stationary producing att [128, q] where
    v_aug = [v_h(64) | ones(64)] so rows 64-127 carry the rowsum replicated
    64x -> a single DVE reciprocal yields the normalization broadcast
    (no gpsimd partition_broadcast on the band-boundary critical path).
  - normalization DVE muls scatter normalized attn DIRECTLY into the o_proj
    "pair" layout: partition p = (w%2)*64 + d, column = (w//2)*128 + u for
    output row u, with q = u*16 + w. o_proj then runs K=128 matmuls (two
    w-blocks per MM) against untouched Wo row-chunks, halving o_proj columns.
  - PSUM: 4 score banks (2 tiles x 2) + 2 att + 2 accumulator = 8.
  - software-pipelined emission: bands(b) | proj(b+1) | ... with ALL o_proj
    emitted last as PE filler-of-last-resort; per-ec xt tiles (bufs=3) and
    half-tile first-batch DMA pieces (transfer-bound, not HWDGE-bound) keep
    next-batch projection chains ready to fill the ACT-bound attention gaps.
  - DMA ordering is the schedule: wqk | xt(0) | wv | xt(1) | wo+boeff |
    xt(2) | ... on the SP ring; tiny consts on the Pool/SWDGE ring.

HW rules learned by probing (CoreSim accepts all of these, HW does not):
  - matmuls from DIFFERENT PE row groups must not write the same PSUM bank
    (same-row-group region-sharing of a bank is fine).
  - column-positioned matmuls (tile_position=(0,32j), PSUM output at a
    partition offset) mis-execute. Matmul lhsT/rhs share their SBUF base
    partition. DVE ops MAY write partition-shifted outputs.
"""

import sys

if "/opt/trn_rl_repo" not in sys.path:
    sys.path.insert(0, "/opt/trn_rl_repo")

import numpy as np

B, S, E, H = 4, 2048, 1024, 16
D = E // H          # 64
NCORES = 8
HPC = H // NCORES   # heads per core = 2
SCALE = 1.0 / float(np.sqrt(D))
NB = S // 256       # 8 bands of 256 queries
NC = S // 128       # 16 key chunks

_CACHE = {}


def _build_program():
    import concourse.bass as bass  # noqa: F401
    import concourse.tile as tile
    from concourse import bacc, mybir

    f16 = mybir.dt.float16
    f32 = mybir.dt.float32
    Exp = mybir.ActivationFunctionType.Exp

    nc = bacc.Bacc("TRN2", target_bir_lowering=False, debug=False)

    xT = nc.dram_tensor("xT", [B, E, S], f16, kind="ExternalInput")
    wqk = nc.dram_tensor("wqk", [E, 256], f16, kind="ExternalInput")
    wv = nc.dram_tensor("wv", [E, 128], f16, kind="ExternalInput")
    bqk = nc.dram_tensor("bqk", [128, 2], f32, kind="ExternalInput")
    wo = nc.dram_tensor("wo", [E, E], f16, kind="ExternalInput")
    boeff = nc.dram_tensor("boeff", [128, 2 * E], f32, kind="ExternalInput")
    trimask2 = nc.dram_tensor("trimask2", [128, 256], f16, kind="ExternalInput")
    out = nc.dram_tensor("out", [B, HPC, 128, E], f32, kind="ExternalOutput")

    with tile.TileContext(nc) as tc:
        with (
            tc.tile_pool(name="const", bufs=1) as cp,
            tc.tile_pool(name="sb", bufs=2) as sb,
            tc.tile_pool(name="sb3", bufs=3) as sb3,
            tc.tile_pool(name="ps", bufs=2, space="PSUM") as ps,
        ):
            # ---- constants resident in SBUF for the whole kernel ----
            # ec0 slice first (364ns) so the first matmul unblocks early;
            # the remainder queues behind the first xt piece
            wqk_sb = cp.tile([128, 8 * 256], f16)     # [p, ec*256 + col]
            wqk_dram = wqk.ap().rearrange("(ec p) c -> p ec c", p=128)
            nc.sync.dma_start(wqk_sb[:, 0:256], wqk_dram[:, 0])
            # wv/bqk/trimask DMAs are deferred until after xt(0) (see below)
            wv_sb = cp.tile([128, 8 * 128], f16)      # [p, ec*128 + vcol]
            bqk_sb = cp.tile([128, 2], f32)
            trimask_sb = cp.tile([128, 256], f16)
            # o_proj constants allocated here, DMA'd after the prologue so
            # they don't delay xt(0) on the shared DMA engines
            wo_sb = cp.tile([128, 8 * E], f16)        # [p, j*1024 + c]
            boeff_sb = cp.tile([128, 2 * E], f32)     # [p, h*1024 + c], bcast rows

            # persistent double-buffered v tiles: per s-chunk c the 256-col
            # group [v_h0(64) | ones(64) | v_h1(64) | ones(64)]; the 64-wide
            # ones blocks make PV emit the rowsum replicated on partitions
            # 64-127 (reciprocal then yields the broadcast directly).
            v2t = [cp.tile([128, NC * 256], f16, name=f"v2_{i}") for i in range(2)]
            for t in v2t:
                tv = t.rearrange("p (c h z) -> p c h z", c=NC, h=2)
                nc.gpsimd.memset(tv[:, :, :, 64:128], 1.0)

            # ================= software-pipelined batch emission ==========
            # order per batch b:  bands(b) | proj(b+1) fillers | o_proj(b)
            # Emitting proj(b+1) BEFORE o_proj(b) keeps the "acc" PSUM slots
            # available to ready filler chains during attention(b) instead of
            # being grabbed by o_proj tiles that are still blocked on the
            # band-3 normalization (slot-allocation priority inversion).
            def emit_xt_dma(b, fine=False, wqk_rest=None):
                # 8 per-ec tiles so projection chains start as each DMA lands.
                # fine=True (first batch): split per (n-block, ec) so the 2
                # "acc"-bank chains complete incrementally from ~3us.
                xts = [
                    sb.tile([128, S], f16, tag=f"xt{ec}", name=f"xt{ec}", bufs=3)
                    for ec in range(8)
                ]
                xt_dram = xT.ap()[b].rearrange("(ec p) s -> p ec s", p=128)
                if fine:
                    # halves: 728ns transfer > 625ns HWDGE overhead, so the
                    # piece stream stays transfer-bound (32-way splits were
                    # HWDGE-bound and stretched xt(0) to ~21us)
                    for n in range(2):
                        for ec in range(8):
                            nc.sync.dma_start(
                                xts[ec][:, n * 1024 : (n + 1) * 1024],
                                xt_dram[:, ec, n * 1024 : (n + 1) * 1024],
                            )
                            if wqk_rest is not None and ec == 0 and n == 0:
                                wsb, wdr = wqk_rest
                                nc.sync.dma_start(
                                    wsb.rearrange("p (ec c) -> p ec c", ec=8)[
                                        :, 1:8
                                    ],
                                    wdr[:, 1:8],
                                )
                else:
                    for ec in range(8):
                        nc.sync.dma_start(xts[ec], xt_dram[:, ec])
                return xts

            def emit_qkv(b, xts):
                # q/k projection: qkT[d2, s], head-major
                # m=0 -> [q_h0|q_h1] on partitions, m=1 -> [k_h0|k_h1]
                # prologue (b=0): borrow the still-idle att/sc PSUM tags so 6
                # chains progress concurrently as the fine xt pieces land
                tags = ["att", "sc", "acc"] if b == 0 else ["acc"]
                qkT_sb = sb.tile([128, 2 * S], f16, tag="qkT", name="qkT")
                for n in range(S // 512):
                    for m in range(2):
                        pq = ps.tile(
                            [128, 512], f32,
                            tag=tags[(n * 2 + m) % len(tags)], name="pq",
                        )
                        for ec in range(8):
                            nc.tensor.matmul(
                                pq,
                                wqk_sb[:, ec * 256 + m * 128 : ec * 256 + (m + 1) * 128],
                                xts[ec][:, n * 512 : (n + 1) * 512],
                                start=(ec == 0),
                                stop=(ec == 7),
                            )
                        nc.vector.tensor_scalar_add(
                            qkT_sb[:, m * S + n * 512 : m * S + (n + 1) * 512],
                            pq,
                            bqk_sb[:, m : m + 1],
                        )
                return qkT_sb

            def emit_v_group(b, xts, sc4, tag="acc"):
                # v in natural [s, d] layout, 4 s-chunks per PSUM bank
                v2_sb = v2t[b % 2]
                vq = ps.tile([128, 512], f32, tag=tag, name="vq")
                for sub in range(4):
                    c = sc4 * 4 + sub
                    for ec in range(8):
                        nc.tensor.matmul(
                            vq[:, sub * 128 : (sub + 1) * 128],
                            xts[ec][:, c * 128 : (c + 1) * 128],
                            wv_sb[:, ec * 128 : (ec + 1) * 128],
                            start=(ec == 0),
                            stop=(ec == 7),
                        )
                # copy into v2 chunks (skips the ones columns)
                nc.vector.tensor_copy(
                    v2_sb.rearrange("p (c h z) -> p c h z", c=NC, h=2)[
                        :, sc4 * 4 : sc4 * 4 + 4, :, 0:64
                    ],
                    vq.rearrange("p (c h dd) -> p c h dd", c=4, h=2),
                )

            # ---- attention over 4 bands of 512 queries ----
            # HW rule (probe-verified): matmuls from different PE row
            # groups must not write the same PSUM bank -> the two heads'
            # scores go to the two separate banks of one [128,1024] tile,
            # and each head's att accumulator gets its own bank.
            def emit_band(b, qkT_sb, pair, g):
                v2_sb = v2t[b % 2]
                atts = [
                    ps.tile([128, 512], f32, tag="att", name=f"att{h}", bufs=2)
                    for h in range(2)
                ]
                nkj = 4 * g + 4
                for kj in range(nkj):
                    qo = 128 * max(0, kj - 4 * g)
                    scp = ps.tile([128, 1024], f32, tag="sc", name="scp", bufs=2)
                    ex = sb3.tile([128, 1024], f16, tag="ex", name="ex")
                    for h in range(2):
                        nc.tensor.matmul(
                            scp[:, h * 512 + qo : (h + 1) * 512],
                            qkT_sb[h * 64 : (h + 1) * 64,
                                   S + kj * 128 : S + (kj + 1) * 128],
                            qkT_sb[h * 64 : (h + 1) * 64,
                                   g * 512 + qo : (g + 1) * 512],
                            start=True,
                            stop=True,
                            tile_position=(h * 64, 0),
                        )
                    nc.scalar.activation(
                        ex.rearrange("p (h q) -> p h q", h=2)[:, :, qo:512],
                        scp.rearrange("p (h q) -> p h q", h=2)[:, :, qo:512],
                        Exp,
                        scale=SCALE,
                    )
                    if kj >= 4 * g:  # diagonal chunk: zero q < k
                        nc.vector.tensor_mul(
                            ex.rearrange("p (h q) -> p h q", h=2)[
                                :, :, qo : qo + 128
                            ],
                            ex.rearrange("p (h q) -> p h q", h=2)[
                                :, :, qo : qo + 128
                            ],
                            trimask_sb.rearrange("p (h q) -> p h q", h=2),
                        )
                    for h in range(2):
                        nc.tensor.matmul(
                            atts[h][:, qo:512],
                            v2_sb[:, kj * 256 + h * 128 : kj * 256 + (h + 1) * 128],
                            ex[:, h * 512 + qo : (h + 1) * 512],
                            start=(kj == 0),
                            stop=(kj == nkj - 1),
                        )
                # normalize + scatter into o_proj pair layout
                for h in range(2):
                    rb = sb.tile([64, 512], f32, tag="rb", name="rb")
                    nc.vector.reciprocal(rb, atts[h][64:128, :])
                    attv = atts[h].rearrange(
                        "p (u2 w2 pr) -> p u2 w2 pr", u2=32, w2=8
                    )
                    rbv = rb.rearrange(
                        "p (u2 w2 pr) -> p u2 w2 pr", u2=32, w2=8
                    )
                    pav = pair[h].rearrange("p (j u) -> p u j", j=8)
                    for par in range(2):
                        nc.vector.tensor_mul(
                            pav[par * 64 : (par + 1) * 64,
                                g * 32 : (g + 1) * 32, :],
                            attv[0:64, :, :, par : par + 1],
                            rbv[0:64, :, :, par : par + 1],
                        )

            def emit_oproj(b, pair):
                # o_proj: po[u, c] = sum_j pair[h][:, j*128:+128]^T wo_j
                # bias added on DVE during the PSUM->SBUF copy
                for h in range(2):
                    out_sb = sb.tile([128, E], f32, tag="osb", name="osb")
                    for n2 in range(2):
                        po = ps.tile([128, 512], f32, tag="acc", name="po")
                        for j in range(8):
                            nc.tensor.matmul(
                                po,
                                pair[h][:, j * 128 : (j + 1) * 128],
                                wo_sb[:, j * E + n2 * 512 : j * E + (n2 + 1) * 512],
                                start=(j == 0),
                                stop=(j == 7),
                            )
                        nc.vector.tensor_add(
                            out_sb[:, n2 * 512 : (n2 + 1) * 512],
                            po,
                            boeff_sb[:, h * E + n2 * 512 : h * E + (n2 + 1) * 512],
                        )
                        # per-half store shortens the kernel-exit tail;
                        # final batch rides the by-then-idle SP ring
                        ring = nc.sync if b == B - 1 else nc.scalar
                        ring.dma_start(
                            out.ap()[b, h, :, n2 * 512 : (n2 + 1) * 512],
                            out_sb[:, n2 * 512 : (n2 + 1) * 512],
                        )

            def emit_proj(b, xts, interleave=False):
                if interleave:  # (measured slower; kept for reference)
                    # prologue: alternate qk-chain pairs and v chains so both
                    # kinds progress as the fine xt(0) pieces land
                    qkT_sb = sb.tile([128, 2 * S], f16, tag="qkT", name="qkT")
                    tags = ["att", "sc", "acc"]
                    ti = 0
                    for n in range(4):
                        for m in range(2):
                            pq = ps.tile([128, 512], f32, tag=tags[ti % 3],
                                         name="pq")
                            ti += 1
                            for ec in range(8):
                                nc.tensor.matmul(
                                    pq,
                                    wqk_sb[:, ec * 256 + m * 128 : ec * 256 + (m + 1) * 128],
                                    xts[ec][:, n * 512 : (n + 1) * 512],
                                    start=(ec == 0),
                                    stop=(ec == 7),
                                )
                            nc.vector.tensor_scalar_add(
                                qkT_sb[:, m * S + n * 512 : m * S + (n + 1) * 512],
                                pq,
                                bqk_sb[:, m : m + 1],
                            )
                        emit_v_group(b, xts, n, tag=tags[ti % 3])
                        ti += 1
                    return qkT_sb
                qkT_sb = emit_qkv(b, xts)
                vtags = ["sc", "att", "acc", "acc"] if b == 0 else ["acc"] * 4
                for sc4 in range(4):
                    emit_v_group(b, xts, sc4, tag=vtags[sc4])
                return qkT_sb

            # prologue: SP DMA queue order is the schedule —
            # wqk | xt(0) fine | xt(1) | wo+boeff | xt(2) ...
            # small consts ride the Pool/SWDGE path, off the HWDGE queue
            nc.gpsimd.dma_start(bqk_sb, bqk.ap())
            nc.gpsimd.dma_start(trimask_sb, trimask2.ap())
            xtss = {0: emit_xt_dma(0, fine=True, wqk_rest=(wqk_sb, wqk_dram))}
            nc.sync.dma_start(
                wv_sb.rearrange("p (ec c) -> p ec c", ec=8),
                wv.ap().rearrange("(ec p) c -> p ec c", p=128),
            )
            qkts = {0: emit_proj(0, xtss.pop(0))}
            xtss[1] = emit_xt_dma(1)
            nc.sync.dma_start(
                wo_sb.rearrange("p (j c) -> p j c", j=8),
                wo.ap().rearrange("(j p) c -> p j c", p=128),
            )
            nc.sync.dma_start(boeff_sb, boeff.ap())
            pairs = {}
            for b in range(B):
                # pair-layout attn tiles: partition (w%2)*64+d, col (w//2)*128+u
                pairs[b] = [
                    sb.tile([128, 8 * 128], f16, tag=f"pair{h}", name=f"pair{h}",
                            bufs=4)
                    for h in range(2)
                ]
                for g in range(4):
                    emit_band(b, qkts[b], pairs[b], g)
                del qkts[b]
                if b + 2 < B:
                    xtss[b + 2] = emit_xt_dma(b + 2)
                if b + 1 < B:
                    qkts[b + 1] = emit_proj(b + 1, xtss.pop(b + 1))
            # o_proj emitted LAST: ready long before its priority comes up,
            # so it acts as PE filler-of-last-resort (esp. the final batch's
            # ACT-bound attention stretch, which has no next-batch filler)
            for b in range(B):
                emit_oproj(b, pairs[b])

    nc.compile()
    return nc


def _get_program():
    if "nc" not in _CACHE:
        _CACHE["nc"] = _build_program()
    return _CACHE["nc"]


def _host_inputs(x, Wqkv, bqkv, Wo, bo):
    """Per-core input maps (host-side layout prep: cast/slice/fold)."""
    xT = np.ascontiguousarray(x.transpose(0, 2, 1)).astype(np.float16)

    wo16 = Wo.astype(np.float16)

    # fold v-bias through attention (softmax rows sum to 1) into o_proj bias:
    # boeff_h = bo + bv_h @ sum_w Wo[w*64+d, :]
    wsum = Wo.reshape(16, 64, E).sum(axis=0)      # [64, E] float32

    k_idx = np.arange(128)[:, None]
    q_idx = np.arange(128)[None, :]
    tri = (k_idx <= q_idx).astype(np.float16)
    trimask2 = np.concatenate([tri, tri], axis=1)  # [128, 256]

    in_maps = []
    for c in range(NCORES):
        h0, h1 = HPC * c, HPC * c + 1
        qcols = list(range(h0 * 3 * D, h0 * 3 * D + 64)) + list(
            range(h1 * 3 * D, h1 * 3 * D + 64)
        )
        kcols = [cc + 64 for cc in qcols]
        vcols = [cc + 128 for cc in qcols]
        bqk_arr = np.stack(
            [bqkv[qcols].astype(np.float32), bqkv[kcols].astype(np.float32)], axis=1
        )  # [128, 2]
        boeff = np.zeros((128, 2 * E), np.float32)
        for i, h in enumerate((h0, h1)):
            bv = bqkv[h * 3 * D + 128 : h * 3 * D + 192].astype(np.float32)
            boeff[:, i * E : (i + 1) * E] = (bo.astype(np.float32) + bv @ wsum)[None, :]
        in_maps.append(
            {
                "xT": xT,
                "wqk": np.ascontiguousarray(Wqkv[:, qcols + kcols]).astype(np.float16),
                "wv": np.ascontiguousarray(Wqkv[:, vcols]).astype(np.float16),
                "bqk": np.ascontiguousarray(bqk_arr),
                "wo": wo16,
                "boeff": boeff,
                "trimask2": trimask2,
            }
        )
    return in_maps


def kernel(x, mask, Wqkv, bqkv, Wo, bo, _n_cores=NCORES, _trace=False):
    """Full-input, full-output MHA. `mask` is the causal tril mask (hardcoded)."""
    from concourse.bass_utils import run_bass_kernel_spmd

    nc = _get_program()
    in_maps = _host_inputs(
        np.asarray(x), np.asarray(Wqkv), np.asarray(bqkv), np.asarray(Wo), np.asarray(bo)
    )[:_n_cores]
    res = run_bass_kernel_spmd(
        nc, in_maps, core_ids=list(range(_n_cores)), trace=_trace
    )
    out_full = np.zeros((B, S, E), np.float32)
    for c in range(_n_cores):
        o = res.results[c]["out"]  # [B, HPC, 128, E]
        for h in range(HPC):
            g = HPC * c + h
            out_full[:, g * 128 : (g + 1) * 128, :] = o[:, h]
    _CACHE["last_results"] = res
    return out_full



# revision 51
# speedup vs baseline: 1.0005x; 1.0005x over previous
"""Trainium2 Bass kernel for nn_MultiHeadAttention_46093589021200.

Causal MHA: B=4, S=2048, E=1024, H=16, D=64, with the reference's
"no-transpose-back" reshape (b,h,s,d)->(b,s,e) before the output projection.

Sharding: pure head-parallel, 2 heads per core, zero collectives.
Because of the reshape quirk, output rows s' in [h*128,(h+1)*128) depend only
on head h, so each core produces two independent 128-row output bands per
batch.

v2 design notes (vs the v1 baseline; 344250ns -> 241624ns):
  - q/k projected via PE into qkT [d2, s] head-major; v projected separately
    in NATURAL [s, d] layout (lhsT = x^T chunk), which is exactly the PV lhsT
    layout -> no DMA xbar transposes at all.
  - v bias folded into an effective o_proj bias on host (softmax rows sum to
    1, so + bv commutes through the attention average); o_proj bias added by
    DVE during the PSUM->SBUF copy (no K=1 bias matmuls).
  - attention in 512-wide q bands: scoresT [k,q] per 128-k chunk, the two
    heads in the two separate banks of one [128,1024] PSUM tile; one exp ACT
    instr per chunk covering both heads; triangular fp16 mask multiply on
    diagonal chunks; PV with v_aug stationary producing att [128, q] where
    v_aug = [v_h(64) | ones(64)] so rows 64-127 carry the rowsum replicated
    64x -> a single DVE reciprocal yields the normalization broadcast
    (no gpsimd partition_broadcast on the band-boundary critical path).
  - normalization DVE muls scatter normalized attn DIRECTLY into the o_proj
    "pair" layout: partition p = (w%2)*64 + d, column = (w//2)*128 + u for
    output row u, with q = u*16 + w. o_proj then runs K=128 matmuls (two
    w-blocks per MM) against untouched Wo row-chunks, halving o_proj columns.
  - PSUM: 4 score banks (2 tiles x 2) + 2 att + 2 accumulator = 8.
  - software-pipelined emission: bands(b) | proj(b+1) | ... with ALL o_proj
    emitted last as PE filler-of-last-resort; per-ec xt tiles (bufs=3) and
    half-tile first-batch DMA pieces (transfer-bound, not HWDGE-bound) keep
    next-batch projection chains ready to fill the ACT-bound attention gaps.
  - DMA ordering is the schedule: wqk | xt(0) | wv | xt(1) | wo+boeff |
    xt(2) | ... on the SP ring; tiny consts on the Pool/SWDGE ring.

HW rules learned by probing (CoreSim accepts all of these, HW does not):
  - matmuls from DIFFERENT PE row groups must not write the same PSUM bank
    (same-row-group region-sharing of a bank is fine).
  - column-positioned matmuls (tile_position=(0,32j), PSUM output at a
    partition offset) mis-execute. Matmul lhsT/rhs share their SBUF base
    partition. DVE ops MAY write partition-shifted outputs.
"""

import sys

if "/opt/trn_rl_repo" not in sys.path:
    sys.path.insert(0, "/opt/trn_rl_repo")

import numpy as np

B, S, E, H = 4, 2048, 1024, 16
D = E // H          # 64
NCORES = 8
HPC = H // NCORES   # heads per core = 2
SCALE = 1.0 / float(np.sqrt(D))
NB = S // 256       # 8 bands of 256 queries
NC = S // 128       # 16 key chunks

_CACHE = {}


def _build_program():
    import concourse.bass as bass  # noqa: F401
    import concourse.tile as tile
    from concourse import bacc, mybir

    f16 = mybir.dt.float16
    f32 = mybir.dt.float32
    Exp = mybir.ActivationFunctionType.Exp

    nc = bacc.Bacc("TRN2", target_bir_lowering=False, debug=False)

    xT = nc.dram_tensor("xT", [B, E, S], f16, kind="ExternalInput")
    wqk = nc.dram_tensor("wqk", [E, 256], f16, kind="ExternalInput")
    wv = nc.dram_tensor("wv", [E, 128], f16, kind="ExternalInput")
    bqk = nc.dram_tensor("bqk", [128, 2], f32, kind="ExternalInput")
    wo = nc.dram_tensor("wo", [E, E], f16, kind="ExternalInput")
    boeff = nc.dram_tensor("boeff", [128, 2 * E], f32, kind="ExternalInput")
    trimask2 = nc.dram_tensor("trimask2", [128, 256], f16, kind="ExternalInput")
    out = nc.dram_tensor("out", [B, HPC, 128, E], f32, kind="ExternalOutput")

    with tile.TileContext(nc) as tc:
        with (
            tc.tile_pool(name="const", bufs=1) as cp,
            tc.tile_pool(name="sb", bufs=2) as sb,
            tc.tile_pool(name="sb3", bufs=3) as sb3,
            tc.tile_pool(name="ps", bufs=2, space="PSUM") as ps,
        ):
            # ---- constants resident in SBUF for the whole kernel ----
            # ec0 slice first (364ns) so the first matmul unblocks early;
            # the remainder queues behind the first xt piece
            wqk_sb = cp.tile([128, 8 * 256], f16)     # [p, ec*256 + col]
            wqk_dram = wqk.ap().rearrange("(ec p) c -> p ec c", p=128)
            nc.sync.dma_start(wqk_sb[:, 0:256], wqk_dram[:, 0])
            # wv/bqk/trimask DMAs are deferred until after xt(0) (see below)
            wv_sb = cp.tile([128, 8 * 128], f16)      # [p, ec*128 + vcol]
            bqk_sb = cp.tile([128, 2], f32)
            trimask_sb = cp.tile([128, 256], f16)
            # o_proj constants allocated here, DMA'd after the prologue so
            # they don't delay xt(0) on the shared DMA engines
            wo_sb = cp.tile([128, 8 * E], f16)        # [p, j*1024 + c]
            boeff_sb = cp.tile([128, 2 * E], f32)     # [p, h*1024 + c], bcast rows

            # persistent double-buffered v tiles: per s-chunk c the 256-col
            # group [v_h0(64) | ones(64) | v_h1(64) | ones(64)]; the 64-wide
            # ones blocks make PV emit the rowsum replicated on partitions
            # 64-127 (reciprocal then yields the broadcast directly).
            v2t = [cp.tile([128, NC * 256], f16, name=f"v2_{i}") for i in range(2)]
            for t in v2t:
                tv = t.rearrange("p (c h z) -> p c h z", c=NC, h=2)
                nc.gpsimd.memset(tv[:, :, :, 64:128], 1.0)

            # ================= software-pipelined batch emission ==========
            # order per batch b:  bands(b) | proj(b+1) fillers | o_proj(b)
            # Emitting proj(b+1) BEFORE o_proj(b) keeps the "acc" PSUM slots
            # available to ready filler chains during attention(b) instead of
            # being grabbed by o_proj tiles that are still blocked on the
            # band-3 normalization (slot-allocation priority inversion).
            def emit_xt_dma(b, fine=False, wqk_rest=None):
                # 8 per-ec tiles so projection chains start as each DMA lands.
                # fine=True (first batch): split per (n-block, ec) so the 2
                # "acc"-bank chains complete incrementally from ~3us.
                xts = [
                    sb.tile([128, S], f16, tag=f"xt{ec}", name=f"xt{ec}", bufs=3)
                    for ec in range(8)
                ]
                xt_dram = xT.ap()[b].rearrange("(ec p) s -> p ec s", p=128)
                if fine:
                    # halves: 728ns transfer > 625ns HWDGE overhead, so the
                    # piece stream stays transfer-bound (32-way splits were
                    # HWDGE-bound and stretched xt(0) to ~21us)
                    for n in range(2):
                        for ec in range(8):
                            nc.sync.dma_start(
                                xts[ec][:, n * 1024 : (n + 1) * 1024],
                                xt_dram[:, ec, n * 1024 : (n + 1) * 1024],
                            )
                            if wqk_rest is not None and ec == 0 and n == 0:
                                wsb, wdr = wqk_rest
                                nc.sync.dma_start(
                                    wsb.rearrange("p (ec c) -> p ec c", ec=8)[
                                        :, 1:8
                                    ],
                                    wdr[:, 1:8],
                                )
                else:
                    for ec in range(8):
                        nc.sync.dma_start(xts[ec], xt_dram[:, ec])
                return xts

            def emit_qkv(b, xts):
                # q/k projection: qkT[d2, s], head-major
                # m=0 -> [q_h0|q_h1] on partitions, m=1 -> [k_h0|k_h1]
                # prologue (b=0): borrow the still-idle att/sc PSUM tags so 6
                # chains progress concurrently as the fine xt pieces land
                tags = ["att", "sc", "acc"] if b == 0 else ["acc"]
                qkT_sb = sb.tile([128, 2 * S], f16, tag="qkT", name="qkT")
                for n in range(S // 512):
                    for m in range(2):
                        pq = ps.tile(
                            [128, 512], f32,
                            tag=tags[(n * 2 + m) % len(tags)], name="pq",
                        )
                        for ec in range(8):
                            nc.tensor.matmul(
                                pq,
                                wqk_sb[:, ec * 256 + m * 128 : ec * 256 + (m + 1) * 128],
                                xts[ec][:, n * 512 : (n + 1) * 512],
                                start=(ec == 0),
                                stop=(ec == 7),
                            )
                        nc.vector.tensor_scalar_add(
                            qkT_sb[:, m * S + n * 512 : m * S + (n + 1) * 512],
                            pq,
                            bqk_sb[:, m : m + 1],
                        )
                return qkT_sb

            def emit_v_group(b, xts, sc4, tag="acc"):
                # v in natural [s, d] layout, 4 s-chunks per PSUM bank
                v2_sb = v2t[b % 2]
                vq = ps.tile([128, 512], f32, tag=tag, name="vq")
                for sub in range(4):
                    c = sc4 * 4 + sub
                    for ec in range(8):
                        nc.tensor.matmul(
                            vq[:, sub * 128 : (sub + 1) * 128],
                            xts[ec][:, c * 128 : (c + 1) * 128],
                            wv_sb[:, ec * 128 : (ec + 1) * 128],
                            start=(ec == 0),
                            stop=(ec == 7),
                        )
                # copy into v2 chunks (skips the ones columns)
                nc.vector.tensor_copy(
                    v2_sb.rearrange("p (c h z) -> p c h z", c=NC, h=2)[
                        :, sc4 * 4 : sc4 * 4 + 4, :, 0:64
                    ],
                    vq.rearrange("p (c h dd) -> p c h dd", c=4, h=2),
                )

            # ---- attention over 4 bands of 512 queries ----
            # HW rule (probe-verified): matmuls from different PE row
            # groups must not write the same PSUM bank -> the two heads'
            # scores go to the two separate banks of one [128,1024] tile,
            # and each head's att accumulator gets its own bank.
            def emit_band(b, qkT_sb, pair, g):
                v2_sb = v2t[b % 2]
                atts = [
                    ps.tile([128, 512], f32, tag="att", name=f"att{h}", bufs=2)
                    for h in range(2)
                ]
                nkj = 4 * g + 4

                def emit_pv(kj, qo, ex):
                    for h in range(2):
                        nc.tensor.matmul(
                            atts[h][:, qo:512],
                            v2_sb[:, kj * 256 + h * 128 : kj * 256 + (h + 1) * 128],
                            ex[:, h * 512 + qo : (h + 1) * 512],
                            start=(kj == 0),
                            stop=(kj == nkj - 1),
                        )

                pend = []
                for kj in range(nkj):
                    qo = 128 * max(0, kj - 4 * g)
                    scp = ps.tile([128, 1024], f32, tag="sc", name="scp", bufs=2)
                    ex = sb3.tile([128, 1024], f16, tag="ex", name="ex")
                    for h in range(2):
                        nc.tensor.matmul(
                            scp[:, h * 512 + qo : (h + 1) * 512],
                            qkT_sb[h * 64 : (h + 1) * 64,
                                   S + kj * 128 : S + (kj + 1) * 128],
                            qkT_sb[h * 64 : (h + 1) * 64,
                                   g * 512 + qo : (g + 1) * 512],
                            start=True,
                            stop=True,
                            tile_position=(h * 64, 0),
                        )
                    nc.scalar.activation(
                        ex.rearrange("p (h q) -> p h q", h=2)[:, :, qo:512],
                        scp.rearrange("p (h q) -> p h q", h=2)[:, :, qo:512],
                        Exp,
                        scale=SCALE,
                    )
                    if kj >= 4 * g:  # diagonal chunk: zero q < k
                        nc.vector.tensor_mul(
                            ex.rearrange("p (h q) -> p h q", h=2)[
                                :, :, qo : qo + 128
                            ],
                            ex.rearrange("p (h q) -> p h q", h=2)[
                                :, :, qo : qo + 128
                            ],
                            trimask_sb.rearrange("p (h q) -> p h q", h=2),
                        )
                    # off-diagonal PV trails scores by one chunk so the exp
                    # ACT + sem propagation has settled when PE reaches it;
                    # diag chunks stay immediate so the band-end
                    # normalization chain is not delayed
                    pend.append((kj, qo, ex))
                    if kj >= 4 * g - 1:
                        while pend:
                            emit_pv(*pend.pop(0))
                    elif len(pend) > 1:
                        emit_pv(*pend.pop(0))
                # normalize + scatter into o_proj pair layout
                for h in range(2):
                    rb = sb.tile([64, 512], f32, tag="rb", name="rb")
                    nc.vector.reciprocal(rb, atts[h][64:128, :])
                    attv = atts[h].rearrange(
                        "p (u2 w2 pr) -> p u2 w2 pr", u2=32, w2=8
                    )
                    rbv = rb.rearrange(
                        "p (u2 w2 pr) -> p u2 w2 pr", u2=32, w2=8
                    )
                    pav = pair[h].rearrange("p (j u) -> p u j", j=8)
                    for par in range(2):
                        nc.vector.tensor_mul(
                            pav[par * 64 : (par + 1) * 64,
                                g * 32 : (g + 1) * 32, :],
                            attv[0:64, :, :, par : par + 1],
                            rbv[0:64, :, :, par : par + 1],
                        )

            def emit_oproj(b, pair):
                # o_proj: po[u, c] = sum_j pair[h][:, j*128:+128]^T wo_j
                # bias added on DVE during the PSUM->SBUF copy
                for h in range(2):
                    out_sb = sb.tile([128, E], f32, tag="osb", name="osb")
                    for n2 in range(2):
                        po = ps.tile([128, 512], f32, tag="acc", name="po")
                        for j in range(8):
                            nc.tensor.matmul(
                                po,
                                pair[h][:, j * 128 : (j + 1) * 128],
                                wo_sb[:, j * E + n2 * 512 : j * E + (n2 + 1) * 512],
                                start=(j == 0),
                                stop=(j == 7),
                            )
                        nc.vector.tensor_add(
                            out_sb[:, n2 * 512 : (n2 + 1) * 512],
                            po,
                            boeff_sb[:, h * E + n2 * 512 : h * E + (n2 + 1) * 512],
                        )
                        # per-half store shortens the kernel-exit tail;
                        # final batch rides the by-then-idle SP ring
                        ring = nc.sync if b == B - 1 else nc.scalar
                        ring.dma_start(
                            out.ap()[b, h, :, n2 * 512 : (n2 + 1) * 512],
                            out_sb[:, n2 * 512 : (n2 + 1) * 512],
                        )

            def emit_proj(b, xts, interleave=False):
                if interleave:  # (measured slower; kept for reference)
                    # prologue: alternate qk-chain pairs and v chains so both
                    # kinds progress as the fine xt(0) pieces land
                    qkT_sb = sb.tile([128, 2 * S], f16, tag="qkT", name="qkT")
                    tags = ["att", "sc", "acc"]
                    ti = 0
                    for n in range(4):
                        for m in range(2):
                            pq = ps.tile([128, 512], f32, tag=tags[ti % 3],
                                         name="pq")
                            ti += 1
                            for ec in range(8):
                                nc.tensor.matmul(
                                    pq,
                                    wqk_sb[:, ec * 256 + m * 128 : ec * 256 + (m + 1) * 128],
                                    xts[ec][:, n * 512 : (n + 1) * 512],
                                    start=(ec == 0),
                                    stop=(ec == 7),
                                )
                            nc.vector.tensor_scalar_add(
                                qkT_sb[:, m * S + n * 512 : m * S + (n + 1) * 512],
                                pq,
                                bqk_sb[:, m : m + 1],
                            )
                        emit_v_group(b, xts, n, tag=tags[ti % 3])
                        ti += 1
                    return qkT_sb
                qkT_sb = emit_qkv(b, xts)
                vtags = ["sc", "att", "acc", "acc"] if b == 0 else ["acc"] * 4
                for sc4 in range(4):
                    emit_v_group(b, xts, sc4, tag=vtags[sc4])
                return qkT_sb

            # prologue: SP DMA queue order is the schedule —
            # wqk | xt(0) fine | xt(1) | wo+boeff | xt(2) ...
            # small consts ride the Pool/SWDGE path, off the HWDGE queue
            nc.gpsimd.dma_start(bqk_sb, bqk.ap())
            nc.gpsimd.dma_start(trimask_sb, trimask2.ap())
            xtss = {0: emit_xt_dma(0, fine=True, wqk_rest=(wqk_sb, wqk_dram))}
            nc.sync.dma_start(
                wv_sb.rearrange("p (ec c) -> p ec c", ec=8),
                wv.ap().rearrange("(ec p) c -> p ec c", p=128),
            )
            qkts = {0: emit_proj(0, xtss.pop(0))}
            xtss[1] = emit_xt_dma(1)
            nc.sync.dma_start(
                wo_sb.rearrange("p (j c) -> p j c", j=8),
                wo.ap().rearrange("(j p) c -> p j c", p=128),
            )
            nc.sync.dma_start(boeff_sb, boeff.ap())
            pairs = {}
            for b in range(B):
                # pair-layout attn tiles: partition (w%2)*64+d, col (w//2)*128+u
                pairs[b] = [
                    sb.tile([128, 8 * 128], f16, tag=f"pair{h}", name=f"pair{h}",
                            bufs=4)
                    for h in range(2)
                ]
                for g in range(4):
                    emit_band(b, qkts[b], pairs[b], g)
                del qkts[b]
                if b + 2 < B:
                    xtss[b + 2] = emit_xt_dma(b + 2)
                if b + 1 < B:
                    qkts[b + 1] = emit_proj(b + 1, xtss.pop(b + 1))
            # o_proj emitted LAST: ready long before its priority comes up,
            # so it acts as PE filler-of-last-resort (esp. the final batch's
            # ACT-bound attention stretch, which has no next-batch filler)
            for b in range(B):
                emit_oproj(b, pairs[b])

    nc.compile()
    return nc


def _get_program():
    if "nc" not in _CACHE:
        _CACHE["nc"] = _build_program()
    return _CACHE["nc"]


def _host_inputs(x, Wqkv, bqkv, Wo, bo):
    """Per-core input maps (host-side layout prep: cast/slice/fold)."""
    xT = np.ascontiguousarray(x.transpose(0, 2, 1)).astype(np.float16)

    wo16 = Wo.astype(np.float16)

    # fold v-bias through attention (softmax rows sum to 1) into o_proj bias:
    # boeff_h = bo + bv_h @ sum_w Wo[w*64+d, :]
    wsum = Wo.reshape(16, 64, E).sum(axis=0)      # [64, E] float32

    k_idx = np.arange(128)[:, None]
    q_idx = np.arange(128)[None, :]
    tri = (k_idx <= q_idx).astype(np.float16)
    trimask2 = np.concatenate([tri, tri], axis=1)  # [128, 256]

    in_maps = []
    for c in range(NCORES):
        h0, h1 = HPC * c, HPC * c + 1
        qcols = list(range(h0 * 3 * D, h0 * 3 * D + 64)) + list(
            range(h1 * 3 * D, h1 * 3 * D + 64)
        )
        kcols = [cc + 64 for cc in qcols]
        vcols = [cc + 128 for cc in qcols]
        bqk_arr = np.stack(
            [bqkv[qcols].astype(np.float32), bqkv[kcols].astype(np.float32)], axis=1
        )  # [128, 2]
        boeff = np.zeros((128, 2 * E), np.float32)
        for i, h in enumerate((h0, h1)):
            bv = bqkv[h * 3 * D + 128 : h * 3 * D + 192].astype(np.float32)
            boeff[:, i * E : (i + 1) * E] = (bo.astype(np.float32) + bv @ wsum)[None, :]
        in_maps.append(
            {
                "xT": xT,
                "wqk": np.ascontiguousarray(Wqkv[:, qcols + kcols]).astype(np.float16),
                "wv": np.ascontiguousarray(Wqkv[:, vcols]).astype(np.float16),
                "bqk": np.ascontiguousarray(bqk_arr),
                "wo": wo16,
                "boeff": boeff,
                "trimask2": trimask2,
            }
        )
    return in_maps


def kernel(x, mask, Wqkv, bqkv, Wo, bo, _n_cores=NCORES, _trace=False):
    """Full-input, full-output MHA. `mask` is the causal tril mask (hardcoded)."""
    from concourse.bass_utils import run_bass_kernel_spmd

    nc = _get_program()
    in_maps = _host_inputs(
        np.asarray(x), np.asarray(Wqkv), np.asarray(bqkv), np.asarray(Wo), np.asarray(bo)
    )[:_n_cores]
    res = run_bass_kernel_spmd(
        nc, in_maps, core_ids=list(range(_n_cores)), trace=_trace
    )
    out_full = np.zeros((B, S, E), np.float32)
    for c in range(_n_cores):
        o = res.results[c]["out"]  # [B, HPC, 128, E]
        for h in range(HPC):
            g = HPC * c + h
            out_full[:, g * 128 : (g + 1) * 128, :] = o[:, h]
    _CACHE["last_results"] = res
    return out_full



# revision 55
# speedup vs baseline: 1.0068x; 1.0064x over previous
"""Trainium2 Bass kernel for nn_MultiHeadAttention_46093589021200.

Causal MHA: B=4, S=2048, E=1024, H=16, D=64, with the reference's
"no-transpose-back" reshape (b,h,s,d)->(b,s,e) before the output projection.

Sharding: pure head-parallel, 2 heads per core, zero collectives.
Because of the reshape quirk, output rows s' in [h*128,(h+1)*128) depend only
on head h, so each core produces two independent 128-row output bands per
batch.

v3 design notes (vs v2; 241624ns baseline):
  - PV restructured to put q on PSUM partitions: att2[q,65] accumulates
    exT_chunk[k,q] @ [v_h|ones][k,65] over k-chunks. Cost model charges
    N(=65) per matmul instead of the q-band width (<=512), cutting PV from
    17408 to 8840 cycles per head-batch. The single ones column accumulates
    the softmax row-sum (replacing v2's 64-wide ones block).
  - softmax normalization becomes a per-partition scalar op: reciprocal of
    att2[:,64] then tensor_scalar_mul -> norm [q, (h,d)] f16 in SBUF.
  - o_proj needs (w,d)-on-partitions, so one PE transpose (via identity
    matmul) per 128-q-chunk converts norm [q,128] -> [128,q] f16 in a
    bitcast region of the att2 PSUM tile (banks are exactly full: 2x sc
    [128,1024] + att2 [128,1024] + 2x acc [128,512] = 8 banks). DVE then
    scatters into the o_proj "pair" layout (partition (w%2)*64+d, col
    (w//2)*128+u, with q = u*16+w).
  - with PV halved, attention bands are locally ACT(exp)-bound, and the PE
    executes in order -- so next-batch projection chains and o_proj(b-1) are
    interleaved at CHUNK granularity via a filler-generator queue (v2's
    en-bloc emission after bands would stall behind exp sems). PV for chunk
    kj is emitted after scores(kj+1) so the exp(kj) sem has settled.
  - per-core PE cycles: qk 131072 + v 65536 + scores 139264 + pv 70720 +
    transpose 8192 + o_proj 65536 = 480320 (~200.2us at 2.4GHz).

HW rules learned by probing (CoreSim accepts all of these, HW does not):
  - matmuls from DIFFERENT PE row groups must not write the same PSUM bank
    (same-row-group region-sharing of a bank is fine).
  - column-positioned matmuls (tile_position=(0,32j), PSUM output at a
    partition offset) mis-execute. Matmul lhsT/rhs share their SBUF base
    partition. DVE ops MAY write partition-shifted outputs.
"""

import sys

if "/opt/trn_rl_repo" not in sys.path:
    sys.path.insert(0, "/opt/trn_rl_repo")

import numpy as np

B, S, E, H = 4, 2048, 1024, 16
D = E // H          # 64
NCORES = 8
HPC = H // NCORES   # heads per core = 2
SCALE = 1.0 / float(np.sqrt(D))
NC = S // 128       # 16 key chunks

_CACHE = {}

PE_NS = 1.0 / 2.4   # ns per streamed output column at full pstate
ACT_NS = 1.0 / 1.2  # ns per lane-element on the activation engine


class Filler:
    """Two-priority queue of emission generators, drained in cost-budgeted
    slices.

    Generators emit instructions between yields; each yield value is the
    PE-cost (ns) of what was just emitted. Band emission pulls from this
    queue to keep the in-order PE stream fed through ACT-bound stretches.
    proj generators (hard emission deadline: before the next batch's bands)
    drain first; oproj generators are deliberately held back so the final
    batch -- which has no next-batch projection -- still has PE filler.
    """

    def __init__(self):
        self.projq = []
        self.oprojq = []
        self.pe = 0.0    # cumulative PE ns emitted (bands + fillers)
        self.act = 0.0   # cumulative ACT ns emitted

    def add(self, gen, proj=True):
        (self.projq if proj else self.oprojq).append(gen)

    def drain_until(self, target):
        """Pull filler until cumulative emitted PE work reaches target."""
        while self.pe < target:
            q = self.projq if self.projq else self.oprojq
            if not q:
                return
            try:
                self.pe += max(next(q[0]), 1.0)
            except StopIteration:
                q.pop(0)

    def drain_min(self, ns):
        """Pull ~ns of filler, but never run the emitted-PE clock more than
        ~1.2us ahead of the ACT clock (over-pulling here exhausts the oproj
        reserve before the last batch needs it)."""
        self.drain_until(min(self.pe + ns, self.act + 1200))

    def drain_gen(self, gen):
        """Force-finish one generator (emission-order deadline)."""
        if gen in self.projq:
            self.projq.remove(gen)
        for c in gen:
            self.pe += max(c, 1.0)

    def drain_all(self):
        for q in (self.projq, self.oprojq):
            while q:
                g = q.pop(0)
                for c in g:
                    self.pe += max(c, 1.0)


def _build_program():
    import concourse.bass as bass  # noqa: F401
    import concourse.tile as tile
    from concourse import bacc, mybir

    f16 = mybir.dt.float16
    f32 = mybir.dt.float32
    Exp = mybir.ActivationFunctionType.Exp

    nc = bacc.Bacc("TRN2", target_bir_lowering=False, debug=False)

    xT = nc.dram_tensor("xT", [B, E, S], f16, kind="ExternalInput")
    wqk = nc.dram_tensor("wqk", [E, 256], f16, kind="ExternalInput")
    wv = nc.dram_tensor("wv", [E, 128], f16, kind="ExternalInput")
    bqk = nc.dram_tensor("bqk", [128, 2], f32, kind="ExternalInput")
    wo = nc.dram_tensor("wo", [E, E], f16, kind="ExternalInput")
    boeff = nc.dram_tensor("boeff", [128, 2 * E], f32, kind="ExternalInput")
    trimask2 = nc.dram_tensor("trimask2", [128, 256], f16, kind="ExternalInput")
    eye = nc.dram_tensor("eye", [128, 128], f16, kind="ExternalInput")
    out = nc.dram_tensor("out", [B, HPC, 128, E], f32, kind="ExternalOutput")

    with tile.TileContext(nc) as tc:
        with (
            tc.tile_pool(name="const", bufs=1) as cp,
            tc.tile_pool(name="sb", bufs=2) as sb,
            tc.tile_pool(name="sb3", bufs=3) as sb3,
            tc.tile_pool(name="ps", bufs=2, space="PSUM") as ps,
        ):
            # ---- constants resident in SBUF for the whole kernel ----
            # ec0 slice first (364ns) so the first matmul unblocks early;
            # the remainder queues behind the first xt piece
            wqk_sb = cp.tile([128, 8 * 256], f16)     # [p, ec*256 + col]
            wqk_dram = wqk.ap().rearrange("(ec p) c -> p ec c", p=128)
            nc.sync.dma_start(wqk_sb[:, 0:256], wqk_dram[:, 0])
            # wv/bqk/trimask DMAs are deferred until after xt(0) (see below)
            wv_sb = cp.tile([128, 8 * 128], f16)      # [p, ec*128 + vcol]
            bqk_sb = cp.tile([128, 2], f32)
            trimask_sb = cp.tile([128, 256], f16)
            eye_sb = cp.tile([128, 128], f16)
            # o_proj constants allocated here, DMA'd after the prologue so
            # they don't delay xt(0) on the shared DMA engines
            wo_sb = cp.tile([128, 8 * E], f16)        # [p, j*1024 + c]
            boeff_sb = cp.tile([128, 2 * E], f32)     # [p, h*1024 + c], bcast rows

            # persistent double-buffered v tiles: per s-chunk c the 130-col
            # group [v_h0(64) | one | v_h1(64) | one]; the single ones column
            # accumulates the softmax row-sum during PV.
            # v2-style 256-stride v_aug tiles [v_h0|ones(64)|v_h1|ones(64)]:
            # 256-aligned moving-operand offsets (the earlier 130-stride
            # layout put PV rhs at odd 130-byte offsets, which mis-executes
            # on HW), ones col 64 doubles as the v3-band rowsum column and
            # the 64-wide block serves the v2-style last-batch band.
            v3t = [cp.tile([128, NC * 256], f16, name=f"v3_{i}") for i in range(2)]
            for t in v3t:
                tv = t.rearrange("p (c h z) -> p c h z", c=NC, h=2)
                nc.gpsimd.memset(tv[:, :, :, 64:128], 1.0)

            def emit_xt_dma(b, fine=False, wqk_rest=None):
                # 8 per-ec tiles so projection chains start as each DMA lands.
                # fine=True (first batch): split per (n-block, ec) so the
                # prologue chains complete incrementally from ~3us.
                xts = [
                    sb.tile([128, S], f16, tag=f"xt{ec}", name=f"xt{ec}", bufs=3)
                    for ec in range(8)
                ]
                xt_dram = xT.ap()[b].rearrange("(ec p) s -> p ec s", p=128)
                if fine:
                    # halves: 728ns transfer > 625ns HWDGE overhead, so the
                    # piece stream stays transfer-bound. The very first piece
                    # is a 512-col quarter: exactly what the first projection
                    # matmul needs, so it unblocks ~360ns earlier.
                    for n in range(2):
                        for ec in range(8):
                            if ec == 0 and n == 0:
                                nc.sync.dma_start(
                                    xts[0][:, 0:512], xt_dram[:, 0, 0:512]
                                )
                                nc.sync.dma_start(
                                    xts[0][:, 512:1024], xt_dram[:, 0, 512:1024]
                                )
                            else:
                                nc.sync.dma_start(
                                    xts[ec][:, n * 1024 : (n + 1) * 1024],
                                    xt_dram[:, ec, n * 1024 : (n + 1) * 1024],
                                )
                            if wqk_rest is not None and ec == 0 and n == 0:
                                wsb, wdr = wqk_rest
                                nc.sync.dma_start(
                                    wsb.rearrange("p (ec c) -> p ec c", ec=8)[
                                        :, 1:8
                                    ],
                                    wdr[:, 1:8],
                                )
                else:
                    for ec in range(8):
                        nc.sync.dma_start(xts[ec], xt_dram[:, ec])
                return xts

            def emit_v_group(b, xts, sc4, tag="acc", bufs=2):
                # v in natural [s, d] layout, 4 s-chunks per PSUM bank
                v3_sb = v3t[b % 2]
                vq = ps.tile([128, 512], f32, tag=tag, name="vq", bufs=bufs)
                for sub in range(4):
                    c = sc4 * 4 + sub
                    for ec in range(8):
                        nc.tensor.matmul(
                            vq[:, sub * 128 : (sub + 1) * 128],
                            xts[ec][:, c * 128 : (c + 1) * 128],
                            wv_sb[:, ec * 128 : (ec + 1) * 128],
                            start=(ec == 0),
                            stop=(ec == 7),
                        )
                # copy into v3 chunks (skips the ones columns)
                nc.vector.tensor_copy(
                    v3_sb.rearrange("p (c h z) -> p c h z", c=NC, h=2)[
                        :, sc4 * 4 : sc4 * 4 + 4, :, 0:64
                    ],
                    vq.rearrange("p (c h dd) -> p c h dd", c=4, h=2),
                )

            def emit_proj0(b, xts):
                # prologue-only en-bloc projection (DMA-paced anyway):
                # q/k chains then v groups, borrowing the still-idle PSUM
                # tags so several chains progress as the fine pieces land.
                # only n=0,1 here: the n=2,3 chains are deferred into the
                # filler queue as READY work for bands(0), whose natural
                # filler (proj(1)) is blocked on the xt(1) DMA until ~26us.
                tags = [("att2", 1), ("sc", 2), ("acc", 2)]
                qkT_sb = sb.tile([128, 2 * S], f16, tag="qkT", name="qkT")
                for n in range(2):
                    for m in range(2):
                        tg, bf = tags[(n * 2 + m) % 3]
                        pq = ps.tile([128, 512], f32, tag=tg, name="pq", bufs=bf)
                        for ec in range(8):
                            nc.tensor.matmul(
                                pq,
                                wqk_sb[:, ec * 256 + m * 128 : ec * 256 + (m + 1) * 128],
                                xts[ec][:, n * 512 : (n + 1) * 512],
                                start=(ec == 0),
                                stop=(ec == 7),
                            )
                        nc.vector.tensor_scalar_add(
                            qkT_sb[:, m * S + n * 512 : m * S + (n + 1) * 512],
                            pq,
                            bqk_sb[:, m : m + 1],
                        )
                vtags = [("sc", 2), ("att2", 1)]
                for sc4 in range(2):
                    tg, bf = vtags[sc4]
                    emit_v_group(b, xts, sc4, tag=tg, bufs=bf)
                return qkT_sb

            def proj_steps(b, xts, qkT_sb, ns=range(4)):
                # filler generator: q/k chain pair then v group, per n-block
                # (matches the order bands(b) consume them: band g needs
                # qk n<=g, k all ... emitted ascending; v groups 0..g).
                for n in ns:
                    for m in range(2):
                        pq = ps.tile([128, 512], f32, tag="acc", name="pq")
                        for ec in range(8):
                            nc.tensor.matmul(
                                pq,
                                wqk_sb[:, ec * 256 + m * 128 : ec * 256 + (m + 1) * 128],
                                xts[ec][:, n * 512 : (n + 1) * 512],
                                start=(ec == 0),
                                stop=(ec == 7),
                            )
                            yield 512 * PE_NS
                        nc.vector.tensor_scalar_add(
                            qkT_sb[:, m * S + n * 512 : m * S + (n + 1) * 512],
                            pq,
                            bqk_sb[:, m : m + 1],
                        )
                        yield 0.0
                    v_sb = v3t[b % 2]
                    vq = ps.tile([128, 512], f32, tag="acc", name="vq")
                    for sub in range(4):
                        c = n * 4 + sub
                        for ec in range(8):
                            nc.tensor.matmul(
                                vq[:, sub * 128 : (sub + 1) * 128],
                                xts[ec][:, c * 128 : (c + 1) * 128],
                                wv_sb[:, ec * 128 : (ec + 1) * 128],
                                start=(ec == 0),
                                stop=(ec == 7),
                            )
                            yield 128 * PE_NS
                    nc.vector.tensor_copy(
                        v_sb.rearrange("p (c h z) -> p c h z", c=NC, h=2)[
                            :, n * 4 : n * 4 + 4, :, 0:64
                        ],
                        vq.rearrange("p (c h dd) -> p c h dd", c=4, h=2),
                    )
                    yield 0.0

            def oproj_steps(b, pair, last=False):
                # o_proj: po[u, c] = sum_j pair[h][:, j*128:+128]^T wo_j
                # bias added on DVE during the PSUM->SBUF copy
                for h in range(2):
                    out_sb = sb.tile([128, E], f32, tag="osb", name="osb")
                    for n2 in range(2):
                        po = ps.tile([128, 512], f32, tag="acc", name="po")
                        for j in range(8):
                            nc.tensor.matmul(
                                po,
                                pair[h][:, j * 128 : (j + 1) * 128],
                                wo_sb[:, j * E + n2 * 512 : j * E + (n2 + 1) * 512],
                                start=(j == 0),
                                stop=(j == 7),
                            )
                            yield 512 * PE_NS
                        # all stores ride the SP ring: ACT ring config would
                        # stall the exp pacer; Pool ring SWDGE generation
                        # (~1us/store) would delay the diag mask-muls that
                        # gate PV
                        ring = nc.sync
                        if last and h == 1:
                            # final two chains: 256-col DVE+DMA pieces on
                            # alternating rings pipeline the kernel-exit tail
                            for z in range(2):
                                cl = n2 * 512 + z * 256
                                nc.vector.tensor_add(
                                    out_sb[:, cl : cl + 256],
                                    po[:, z * 256 : (z + 1) * 256],
                                    boeff_sb[:, h * E + cl : h * E + cl + 256],
                                )
                                rg = nc.sync if z == 0 else nc.scalar
                                rg.dma_start(
                                    out.ap()[b, h, :, cl : cl + 256],
                                    out_sb[:, cl : cl + 256],
                                )
                        else:
                            nc.vector.tensor_add(
                                out_sb[:, n2 * 512 : (n2 + 1) * 512],
                                po,
                                boeff_sb[:, h * E + n2 * 512 : h * E + (n2 + 1) * 512],
                            )
                            ring.dma_start(
                                out.ap()[b, h, :, n2 * 512 : (n2 + 1) * 512],
                                out_sb[:, n2 * 512 : (n2 + 1) * 512],
                            )
                        yield 0.0

            # ---- attention over 4 bands of 512 queries ----
            def emit_band2(qkT_sb, pair, g):
                # v2-style band for the LAST batch: PE-dominated per chunk
                # (no filler needed), 64-wide-ones rowsum replication, DVE
                # normalization scatter at band end. atts = the two banks of
                # one att2-tag tile.
                att2_t = ps.tile([128, 1024], f32, tag="att2", name="att2",
                                 bufs=1)
                atts = [att2_t[:, 0:512], att2_t[:, 512:1024]]
                nkj = 4 * g + 4

                def emit_pv2(kj, qo, ex):
                    for h in range(2):
                        nc.tensor.matmul(
                            atts[h][:, qo:512],
                            v3t[1][:, kj * 256 + h * 128 : kj * 256 + (h + 1) * 128],
                            ex[:, h * 512 + qo : (h + 1) * 512],
                            start=(kj == 0),
                            stop=(kj == nkj - 1),
                        )

                pend = []
                for kj in range(nkj):
                    qo = 128 * max(0, kj - 4 * g)
                    scp = ps.tile([128, 1024], f32, tag="sc", name="scp", bufs=2)
                    ex = sb3.tile([128, 1024], f16, tag="ex", name="ex", bufs=5)
                    for h in range(2):
                        nc.tensor.matmul(
                            scp[:, h * 512 + qo : (h + 1) * 512],
                            qkT_sb[h * 64 : (h + 1) * 64,
                                   S + kj * 128 : S + (kj + 1) * 128],
                            qkT_sb[h * 64 : (h + 1) * 64,
                                   g * 512 + qo : (g + 1) * 512],
                            start=True,
                            stop=True,
                            tile_position=(h * 64, 0),
                        )
                    nc.scalar.activation(
                        ex.rearrange("p (h q) -> p h q", h=2)[:, :, qo:512],
                        scp.rearrange("p (h q) -> p h q", h=2)[:, :, qo:512],
                        Exp,
                        scale=SCALE,
                    )
                    if kj >= 4 * g:  # diagonal chunk: zero q < k
                        nc.vector.tensor_mul(
                            ex.rearrange("p (h q) -> p h q", h=2)[
                                :, :, qo : qo + 128
                            ],
                            ex.rearrange("p (h q) -> p h q", h=2)[
                                :, :, qo : qo + 128
                            ],
                            trimask_sb.rearrange("p (h q) -> p h q", h=2),
                        )
                    # off-diagonal PV trails scores by one chunk (exp sem
                    # settled); diag immediate so the band end is not delayed
                    pend.append((kj, qo, ex))
                    if kj >= 4 * g - 1:
                        while pend:
                            emit_pv2(*pend.pop(0))
                    elif len(pend) > 1:
                        emit_pv2(*pend.pop(0))
                # normalize + scatter into o_proj pair layout
                for h in range(2):
                    rb = sb.tile([64, 512], f32, tag="rb", name="rb")
                    nc.vector.reciprocal(rb, atts[h][64:128, :])
                    attv = atts[h].rearrange(
                        "p (u2 w2 pr) -> p u2 w2 pr", u2=32, w2=8
                    )
                    rbv = rb.rearrange(
                        "p (u2 w2 pr) -> p u2 w2 pr", u2=32, w2=8
                    )
                    pav = pair[h].rearrange("p (j u) -> p u j", j=8)
                    for par in range(2):
                        nc.vector.tensor_mul(
                            pav[par * 64 : (par + 1) * 64,
                                g * 32 : (g + 1) * 32, :],
                            attv[0:64, :, :, par : par + 1],
                            rbv[0:64, :, :, par : par + 1],
                        )

            def emit_band(b, qkT_sb, pair, g, fill, fine_tail=False):
                # PSUM deps are tracked at BANK granularity: any read of an
                # accumulating bank serializes later matmul writes to it.
                # So: bank0 = h0 data [0:256) + 4 f16 transpose slots
                # [256:512); bank1 = h1 data [512:768) + all rowsums
                # [768:776). ALL normalization reads happen at band end,
                # after both banks' accumulations have stopped.
                v3_sb = v3t[b % 2]
                att2 = ps.tile([128, 1024], f32, tag="att2", name="att2", bufs=1)
                trv = att2[:, 256:512].bitcast(f16)   # [128, 512] f16

                def emit_pv(kj, lo, ex):
                    cost = 0.0
                    for h in range(2):
                        for qc in range(lo, 4):
                            exs = ex[:, h * 512 + qc * 128 : h * 512 + (qc + 1) * 128]
                            nc.tensor.matmul(
                                att2[:, h * 512 + qc * 64 : h * 512 + (qc + 1) * 64],
                                exs,
                                v3_sb[:, kj * 256 + h * 128 : kj * 256 + h * 128 + 64],
                                start=(kj == 0 and h == 0 and qc == 0),
                                stop=(kj == 4 * g + qc),
                                skip_group_check=True,
                            )
                            # softmax row-sum rides as a 1-column matmul of
                            # the ones column (same stationary ex chunk)
                            nc.tensor.matmul(
                                att2[:, 768 + h * 4 + qc : 768 + h * 4 + qc + 1],
                                exs,
                                v3_sb[:, kj * 256 + h * 128 + 64 : kj * 256 + h * 128 + 65],
                                start=(kj == 0 and h == 0 and qc == 0),
                                stop=(kj == 4 * g + qc),
                                skip_group_check=True,
                            )
                            cost += 65 * PE_NS
                    return cost

                nkj = 4 * g + 4
                pend = []
                for kj in range(nkj):
                    qo = 128 * max(0, kj - 4 * g)
                    lo = max(0, kj - 4 * g)
                    # slot order: PV/filler first, scores LAST — this gives
                    # the scp WAR (vs the exp two chunks back) extra slack
                    if len(pend) > 2:
                        fill.pe += emit_pv(*pend.pop(0))
                    fill.drain_until(fill.act)
                    scp = ps.tile([128, 1024], f32, tag="sc", name="scp", bufs=2)
                    ex = sb3.tile([128, 1024], f16, tag="ex", name="ex", bufs=5)
                    fill.pe += 2 * (512 - qo) * PE_NS
                    for h in range(2):
                        nc.tensor.matmul(
                            scp[:, h * 512 + qo : (h + 1) * 512],
                            qkT_sb[h * 64 : (h + 1) * 64,
                                   S + kj * 128 : S + (kj + 1) * 128],
                            qkT_sb[h * 64 : (h + 1) * 64,
                                   g * 512 + qo : (g + 1) * 512],
                            start=True,
                            stop=True,
                            tile_position=(h * 64, 0),
                        )
                    nc.scalar.activation(
                        ex.rearrange("p (h q) -> p h q", h=2)[:, :, qo:512],
                        scp.rearrange("p (h q) -> p h q", h=2)[:, :, qo:512],
                        Exp,
                        scale=SCALE,
                    )
                    fill.act += 2 * (512 - qo) * ACT_NS + 190
                    if kj >= 4 * g:  # diagonal chunk: zero q < k
                        # on the idle Pool engine: DVE is near-saturated
                        # during bands with norm/scatter/finalize work
                        nc.gpsimd.tensor_mul(
                            ex.rearrange("p (h q) -> p h q", h=2)[
                                :, :, qo : qo + 128
                            ],
                            ex.rearrange("p (h q) -> p h q", h=2)[
                                :, :, qo : qo + 128
                            ],
                            trimask_sb.rearrange("p (h q) -> p h q", h=2),
                        )
                    pend.append((kj, lo, ex))
                while pend:
                    fill.pe += emit_pv(*pend.pop(0))
                    fill.drain_min(400)
                # ---- band-end normalization block ----
                # one recip (all 8 rowsums), 8 muls, 4 transposes, 4
                # batched scatters; bank transitions are single-direction
                # (reads of data banks, then writes to bank0 tr slots)
                rcp = sb.tile([128, 8], f32, tag="rcp", name="rcp", bufs=2)
                nc.vector.reciprocal(rcp, att2[:, 768:776])
                nrms = []
                for qc in range(4):
                    nrm = sb.tile([128, 128], f16, tag="nrm", name="nrm",
                                  bufs=4)
                    for h in range(2):
                        nc.vector.tensor_scalar_mul(
                            nrm[:, h * 64 : (h + 1) * 64],
                            att2[:, h * 512 + qc * 64 : h * 512 + (qc + 1) * 64],
                            rcp[:, h * 4 + qc : h * 4 + qc + 1],
                        )
                    nrms.append(nrm)
                    fill.drain_min(350)
                fill.drain_min(600)
                for qc in range(4):
                    # start=False: land on the region zeroed by the band's
                    # PV start (HW zeroes the whole bank on start=True)
                    nc.tensor.matmul(
                        trv[:, qc * 128 : (qc + 1) * 128], nrms[qc], eye_sb,
                        is_transpose=True, start=False, stop=True,
                        skip_group_check=True,
                    )
                    fill.pe += 128 * PE_NS
                fill.drain_min(500)
                trb = trv.rearrange(
                    "p (qc ul j pr) -> p pr j qc ul", qc=4, ul=8, j=8
                )
                for h in range(2):
                    pav = pair[h].rearrange(
                        "p (j qcg ul) -> p j qcg ul", j=8, qcg=16
                    )
                    for par in range(2):
                        nc.vector.tensor_copy(
                            pav[par * 64 : (par + 1) * 64, :,
                                g * 4 : g * 4 + 4, :],
                            trb[h * 64 : (h + 1) * 64, par],
                        )
                # guard reads: the next band's start=True PVs zero their
                # whole bank (invisible to the range tracker); these trailing
                # DVE reads of cols 0 and 768 give those PVs a WAR dep that
                # orders the zeroing after this band's scatters/muls
                grd = sb.tile([128, 2], f32, tag="grd", name="grd", bufs=2)
                nc.vector.tensor_copy(grd[:, 0:1], att2[:, 0:1])
                nc.vector.tensor_copy(grd[:, 1:2], att2[:, 768:769])
                fill.drain_min(400)

            # ================= prologue ==================================
            # SP DMA queue order is the schedule —
            # wqk | xt(0) fine | wv | xt(1) | wo+boeff | xt(2) ...
            # small consts ride the Pool/SWDGE path, off the HWDGE queue
            nc.gpsimd.dma_start(bqk_sb, bqk.ap())
            nc.gpsimd.dma_start(trimask_sb, trimask2.ap())
            nc.gpsimd.dma_start(eye_sb, eye.ap())
            xts0 = emit_xt_dma(0, fine=True, wqk_rest=(wqk_sb, wqk_dram))
            xtss = {}
            nc.sync.dma_start(
                wv_sb.rearrange("p (ec c) -> p ec c", ec=8),
                wv.ap().rearrange("(ec p) c -> p ec c", p=128),
            )
            qkts = {0: emit_proj0(0, xts0)}
            xtss[1] = emit_xt_dma(1)
            nc.sync.dma_start(
                wo_sb.rearrange("p (j c) -> p j c", j=8),
                wo.ap().rearrange("(j p) c -> p j c", p=128),
            )
            nc.sync.dma_start(boeff_sb, boeff.ap())

            fill = Filler()
            # n=2,3 chains of proj(b) are deferred INTO bands(b): band g
            # only reads qkT/v from chains n<=g. proj(3) fully drains during
            # bands(2) because the v2-style bands(3) consume no filler.
            defer = proj_steps(0, xts0, qkts[0], ns=(2, 3))
            pairs = {}
            for b in range(3):
                pairs[b] = [
                    sb.tile([128, 8 * 128], f16, tag=f"pair{h}", name=f"pair{h}",
                            bufs=4)
                    for h in range(2)
                ]
                if defer is not None:
                    fill.add(defer)
                qkts[b + 1] = sb.tile([128, 2 * S], f16, tag="qkT", name="qkT")
                xts_n = xtss.pop(b + 1)
                if b < 2:
                    proj_gen = proj_steps(b + 1, xts_n, qkts[b + 1], ns=(0, 1))
                    defer_next = proj_steps(b + 1, xts_n, qkts[b + 1],
                                            ns=(2, 3))
                else:
                    proj_gen = proj_steps(3, xts_n, qkts[3], ns=range(4))
                    defer_next = None
                fill.add(proj_gen)
                for g in range(4):
                    if g == 1 and defer is not None:
                        # deadline: band 2 reads qkT/v from n=2; one band
                        # early so the v-copy DVE latency is hidden
                        fill.drain_gen(defer)
                    emit_band(b, qkts[b], pairs[b], g, fill)
                # hard deadline: proj(b+1) emitted before bands(b+1)
                fill.drain_gen(proj_gen)
                fill.add(oproj_steps(b, pairs[b]), proj=False)
                defer = defer_next
                del qkts[b]
                if b + 2 < B:
                    xtss[b + 2] = emit_xt_dma(b + 2)
            # last batch: v2-style self-filling bands (PE-dominated chunks)
            pairs[3] = [
                sb.tile([128, 8 * 128], f16, tag=f"pair{h}", name=f"pair{h}",
                        bufs=4)
                for h in range(2)
            ]
            for g in range(4):
                # ~1.2us of oproj filler covers the 2-chunk exp pipeline
                # fill at each band start
                fill.drain_until(fill.pe + 1200)
                emit_band2(qkts[3], pairs[3], g)
            # remaining o_proj emitted LAST as PE filler-of-last-resort
            fill.drain_all()
            for _ in oproj_steps(3, pairs[3], last=True):
                pass

    nc.compile()
    return nc


def _get_program():
    if "nc" not in _CACHE:
        _CACHE["nc"] = _build_program()
    return _CACHE["nc"]


def _host_inputs(x, Wqkv, bqkv, Wo, bo):
    """Per-core input maps (host-side layout prep: cast/slice/fold)."""
    xT = np.ascontiguousarray(x.transpose(0, 2, 1)).astype(np.float16)

    wo16 = Wo.astype(np.float16)

    # fold v-bias through attention (softmax rows sum to 1) into o_proj bias:
    # boeff_h = bo + bv_h @ sum_w Wo[w*64+d, :]
    wsum = Wo.reshape(16, 64, E).sum(axis=0)      # [64, E] float32

    k_idx = np.arange(128)[:, None]
    q_idx = np.arange(128)[None, :]
    tri = (k_idx <= q_idx).astype(np.float16)
    trimask2 = np.concatenate([tri, tri], axis=1)  # [128, 256]
    eye = np.eye(128, dtype=np.float16)

    in_maps = []
    for c in range(NCORES):
        h0, h1 = HPC * c, HPC * c + 1
        qcols = list(range(h0 * 3 * D, h0 * 3 * D + 64)) + list(
            range(h1 * 3 * D, h1 * 3 * D + 64)
        )
        kcols = [cc + 64 for cc in qcols]
        vcols = [cc + 128 for cc in qcols]
        bqk_arr = np.stack(
            [bqkv[qcols].astype(np.float32), bqkv[kcols].astype(np.float32)], axis=1
        )  # [128, 2]
        boeff = np.zeros((128, 2 * E), np.float32)
        for i, h in enumerate((h0, h1)):
            bv = bqkv[h * 3 * D + 128 : h * 3 * D + 192].astype(np.float32)
            boeff[:, i * E : (i + 1) * E] = (bo.astype(np.float32) + bv @ wsum)[None, :]
        in_maps.append(
            {
                "xT": xT,
                "wqk": np.ascontiguousarray(Wqkv[:, qcols + kcols]).astype(np.float16),
                "wv": np.ascontiguousarray(Wqkv[:, vcols]).astype(np.float16),
                "bqk": np.ascontiguousarray(bqk_arr),
                "wo": wo16,
                "boeff": boeff,
                "trimask2": trimask2,
                "eye": eye,
            }
        )
    return in_maps


def kernel(x, mask, Wqkv, bqkv, Wo, bo, _n_cores=NCORES, _trace=False):
    """Full-input, full-output MHA. `mask` is the causal tril mask (hardcoded)."""
    from concourse.bass_utils import run_bass_kernel_spmd

    nc = _get_program()
    in_maps = _host_inputs(
        np.asarray(x), np.asarray(Wqkv), np.asarray(bqkv), np.asarray(Wo), np.asarray(bo)
    )[:_n_cores]
    res = run_bass_kernel_spmd(
        nc, in_maps, core_ids=list(range(_n_cores)), trace=_trace
    )
    out_full = np.zeros((B, S, E), np.float32)
    for c in range(_n_cores):
        o = res.results[c]["out"]  # [B, HPC, 128, E]
        for h in range(HPC):
            g = HPC * c + h
            out_full[:, g * 128 : (g + 1) * 128, :] = o[:, h]
    _CACHE["last_results"] = res
    return out_full


# revision 56
# speedup vs baseline: 1.0247x; 1.0177x over previous
"""Trainium2 Bass kernel for nn_MultiHeadAttention_46093589021200.

Causal MHA: B=4, S=2048, E=1024, H=16, D=64, with the reference's
"no-transpose-back" reshape (b,h,s,d)->(b,s,e) before the output projection.

Sharding: pure head-parallel, 2 heads per core, zero collectives.
Because of the reshape quirk, output rows s' in [h*128,(h+1)*128) depend only
on head h, so each core produces two independent 128-row output bands per
batch.

v3 design notes (vs v2; 241624ns baseline):
  - PV restructured to put q on PSUM partitions: att2[q,65] accumulates
    exT_chunk[k,q] @ [v_h|ones][k,65] over k-chunks. Cost model charges
    N(=65) per matmul instead of the q-band width (<=512), cutting PV from
    17408 to 8840 cycles per head-batch. The single ones column accumulates
    the softmax row-sum (replacing v2's 64-wide ones block).
  - softmax normalization becomes a per-partition scalar op: reciprocal of
    att2[:,64] then tensor_scalar_mul -> norm [q, (h,d)] f16 in SBUF.
  - o_proj needs (w,d)-on-partitions, so one PE transpose (via identity
    matmul) per 128-q-chunk converts norm [q,128] -> [128,q] f16 in a
    bitcast region of the att2 PSUM tile (banks are exactly full: 2x sc
    [128,1024] + att2 [128,1024] + 2x acc [128,512] = 8 banks). DVE then
    scatters into the o_proj "pair" layout (partition (w%2)*64+d, col
    (w//2)*128+u, with q = u*16+w).
  - with PV halved, attention bands are locally ACT(exp)-bound, and the PE
    executes in order -- so next-batch projection chains and o_proj(b-1) are
    interleaved at CHUNK granularity via a filler-generator queue (v2's
    en-bloc emission after bands would stall behind exp sems). PV for chunk
    kj is emitted after scores(kj+1) so the exp(kj) sem has settled.
  - per-core PE cycles: qk 131072 + v 65536 + scores 139264 + pv 70720 +
    transpose 8192 + o_proj 65536 = 480320 (~200.2us at 2.4GHz).

HW rules learned by probing (CoreSim accepts all of these, HW does not):
  - matmuls from DIFFERENT PE row groups must not write the same PSUM bank
    (same-row-group region-sharing of a bank is fine).
  - column-positioned matmuls (tile_position=(0,32j), PSUM output at a
    partition offset) mis-execute. Matmul lhsT/rhs share their SBUF base
    partition. DVE ops MAY write partition-shifted outputs.
"""

import sys

if "/opt/trn_rl_repo" not in sys.path:
    sys.path.insert(0, "/opt/trn_rl_repo")

import numpy as np

B, S, E, H = 4, 2048, 1024, 16
D = E // H          # 64
NCORES = 8
HPC = H // NCORES   # heads per core = 2
SCALE = 1.0 / float(np.sqrt(D))
NC = S // 128       # 16 key chunks

_CACHE = {}

PE_NS = 1.0 / 2.4   # ns per streamed output column at full pstate
ACT_NS = 1.0 / 1.2  # ns per lane-element on the activation engine


class Filler:
    """Two-priority queue of emission generators, drained in cost-budgeted
    slices.

    Generators emit instructions between yields; each yield value is the
    PE-cost (ns) of what was just emitted. Band emission pulls from this
    queue to keep the in-order PE stream fed through ACT-bound stretches.
    proj generators (hard emission deadline: before the next batch's bands)
    drain first; oproj generators are deliberately held back so the final
    batch -- which has no next-batch projection -- still has PE filler.
    """

    def __init__(self):
        self.projq = []
        self.oprojq = []
        self.pe = 0.0    # cumulative PE ns emitted (bands + fillers)
        self.act = 0.0   # cumulative ACT ns emitted

    def add(self, gen, proj=True):
        (self.projq if proj else self.oprojq).append(gen)

    def drain_until(self, target):
        """Pull filler until cumulative emitted PE work reaches target."""
        while self.pe < target:
            q = self.projq if self.projq else self.oprojq
            if not q:
                return
            try:
                self.pe += max(next(q[0]), 1.0)
            except StopIteration:
                q.pop(0)

    def drain_min(self, ns):
        """Pull ~ns of filler, but never run the emitted-PE clock more than
        ~1.2us ahead of the ACT clock (over-pulling here exhausts the oproj
        reserve before the last batch needs it)."""
        self.drain_until(min(self.pe + ns, self.act + 1200))

    def drain_gen(self, gen):
        """Force-finish one generator (emission-order deadline)."""
        if gen in self.projq:
            self.projq.remove(gen)
        for c in gen:
            self.pe += max(c, 1.0)

    def drain_all(self):
        for q in (self.projq, self.oprojq):
            while q:
                g = q.pop(0)
                for c in g:
                    self.pe += max(c, 1.0)


def _build_program():
    import concourse.bass as bass  # noqa: F401
    import concourse.tile as tile
    from concourse import bacc, mybir

    f16 = mybir.dt.float16
    f32 = mybir.dt.float32
    Exp = mybir.ActivationFunctionType.Exp

    nc = bacc.Bacc("TRN2", target_bir_lowering=False, debug=False)

    xT = nc.dram_tensor("xT", [B, E, S], f16, kind="ExternalInput")
    wqk = nc.dram_tensor("wqk", [E, 256], f16, kind="ExternalInput")
    wv = nc.dram_tensor("wv", [E, 128], f16, kind="ExternalInput")
    bqk = nc.dram_tensor("bqk", [128, 2], f32, kind="ExternalInput")
    wo = nc.dram_tensor("wo", [E, E], f16, kind="ExternalInput")
    boeff = nc.dram_tensor("boeff", [128, 2 * E], f32, kind="ExternalInput")
    trimask2 = nc.dram_tensor("trimask2", [128, 256], f16, kind="ExternalInput")
    eye = nc.dram_tensor("eye", [128, 128], f16, kind="ExternalInput")
    out = nc.dram_tensor("out", [B, HPC, 128, E], f32, kind="ExternalOutput")

    with tile.TileContext(nc) as tc:
        with (
            tc.tile_pool(name="const", bufs=1) as cp,
            tc.tile_pool(name="sb", bufs=2) as sb,
            tc.tile_pool(name="sb3", bufs=3) as sb3,
            tc.tile_pool(name="ps", bufs=2, space="PSUM") as ps,
        ):
            # ---- constants resident in SBUF for the whole kernel ----
            # ec0 slice first (364ns) so the first matmul unblocks early;
            # the remainder queues behind the first xt piece
            wqk_sb = cp.tile([128, 8 * 256], f16)     # [p, ec*256 + col]
            wqk_dram = wqk.ap().rearrange("(ec p) c -> p ec c", p=128)
            nc.sync.dma_start(wqk_sb[:, 0:256], wqk_dram[:, 0])
            # wv/bqk/trimask DMAs are deferred until after xt(0) (see below)
            wv_sb = cp.tile([128, 8 * 128], f16)      # [p, ec*128 + vcol]
            bqk_sb = cp.tile([128, 2], f32)
            trimask_sb = cp.tile([128, 256], f16)
            eye_sb = cp.tile([128, 128], f16)
            # o_proj constants allocated here, DMA'd after the prologue so
            # they don't delay xt(0) on the shared DMA engines
            wo_sb = cp.tile([128, 8 * E], f16)        # [p, j*1024 + c]
            boeff_sb = cp.tile([128, 2 * E], f32)     # [p, h*1024 + c], bcast rows

            # persistent double-buffered v tiles: per s-chunk c the 130-col
            # group [v_h0(64) | one | v_h1(64) | one]; the single ones column
            # accumulates the softmax row-sum during PV.
            # v2-style 256-stride v_aug tiles [v_h0|ones(64)|v_h1|ones(64)]:
            # 256-aligned moving-operand offsets (the earlier 130-stride
            # layout put PV rhs at odd 130-byte offsets, which mis-executes
            # on HW), ones col 64 doubles as the v3-band rowsum column and
            # the 64-wide block serves the v2-style last-batch band.
            v3t = [cp.tile([128, NC * 256], f16, name=f"v3_{i}") for i in range(2)]
            for t in v3t:
                tv = t.rearrange("p (c h z) -> p c h z", c=NC, h=2)
                nc.gpsimd.memset(tv[:, :, :, 64:128], 1.0)

            def emit_xt_dma(b, fine=False, wqk_rest=None):
                # 8 per-ec tiles so projection chains start as each DMA lands.
                # fine=True (first batch): split per (n-block, ec) so the
                # prologue chains complete incrementally from ~3us.
                xts = [
                    sb.tile([128, S], f16, tag=f"xt{ec}", name=f"xt{ec}", bufs=3)
                    for ec in range(8)
                ]
                xt_dram = xT.ap()[b].rearrange("(ec p) s -> p ec s", p=128)
                if fine:
                    # halves: 728ns transfer > 625ns HWDGE overhead, so the
                    # piece stream stays transfer-bound. The very first piece
                    # is a 512-col quarter: exactly what the first projection
                    # matmul needs, so it unblocks ~360ns earlier.
                    for n in range(2):
                        for ec in range(8):
                            if ec == 0 and n == 0:
                                nc.sync.dma_start(
                                    xts[0][:, 0:512], xt_dram[:, 0, 0:512]
                                )
                                nc.sync.dma_start(
                                    xts[0][:, 512:1024], xt_dram[:, 0, 512:1024]
                                )
                            else:
                                nc.sync.dma_start(
                                    xts[ec][:, n * 1024 : (n + 1) * 1024],
                                    xt_dram[:, ec, n * 1024 : (n + 1) * 1024],
                                )
                            if wqk_rest is not None and ec == 0 and n == 0:
                                wsb, wdr = wqk_rest
                                nc.sync.dma_start(
                                    wsb.rearrange("p (ec c) -> p ec c", ec=8)[
                                        :, 1:8
                                    ],
                                    wdr[:, 1:8],
                                )
                else:
                    for ec in range(8):
                        nc.sync.dma_start(xts[ec], xt_dram[:, ec])
                return xts

            def emit_v_group(b, xts, sc4, tag="acc", bufs=2):
                # v in natural [s, d] layout, 4 s-chunks per PSUM bank
                v3_sb = v3t[b % 2]
                vq = ps.tile([128, 512], f32, tag=tag, name="vq", bufs=bufs)
                for sub in range(4):
                    c = sc4 * 4 + sub
                    for ec in range(8):
                        nc.tensor.matmul(
                            vq[:, sub * 128 : (sub + 1) * 128],
                            xts[ec][:, c * 128 : (c + 1) * 128],
                            wv_sb[:, ec * 128 : (ec + 1) * 128],
                            start=(ec == 0),
                            stop=(ec == 7),
                        )
                # copy into v3 chunks (skips the ones columns)
                nc.vector.tensor_copy(
                    v3_sb.rearrange("p (c h z) -> p c h z", c=NC, h=2)[
                        :, sc4 * 4 : sc4 * 4 + 4, :, 0:64
                    ],
                    vq.rearrange("p (c h dd) -> p c h dd", c=4, h=2),
                )

            def emit_proj0(b, xts):
                # prologue-only en-bloc projection (DMA-paced anyway):
                # q/k chains then v groups, borrowing the still-idle PSUM
                # tags so several chains progress as the fine pieces land.
                # only n=0,1 here: the n=2,3 chains are deferred into the
                # filler queue as READY work for bands(0), whose natural
                # filler (proj(1)) is blocked on the xt(1) DMA until ~26us.
                tags = [("att2", 1), ("sc", 2), ("acc", 2)]
                qkT_sb = sb.tile([128, 2 * S], f16, tag="qkT", name="qkT")
                for n in range(2):
                    for m in range(2):
                        tg, bf = tags[(n * 2 + m) % 3]
                        pq = ps.tile([128, 512], f32, tag=tg, name="pq", bufs=bf)
                        for ec in range(8):
                            nc.tensor.matmul(
                                pq,
                                wqk_sb[:, ec * 256 + m * 128 : ec * 256 + (m + 1) * 128],
                                xts[ec][:, n * 512 : (n + 1) * 512],
                                start=(ec == 0),
                                stop=(ec == 7),
                            )
                        nc.vector.tensor_scalar_add(
                            qkT_sb[:, m * S + n * 512 : m * S + (n + 1) * 512],
                            pq,
                            bqk_sb[:, m : m + 1],
                        )
                vtags = [("sc", 2), ("att2", 1)]
                for sc4 in range(2):
                    tg, bf = vtags[sc4]
                    emit_v_group(b, xts, sc4, tag=tg, bufs=bf)
                return qkT_sb

            def proj_steps(b, xts, qkT_sb, ns=range(4)):
                # filler generator: q/k chain pair then v group, per n-block
                # (matches the order bands(b) consume them: band g needs
                # qk n<=g, k all ... emitted ascending; v groups 0..g).
                for n in ns:
                    for m in range(2):
                        pq = ps.tile([128, 512], f32, tag="acc", name="pq")
                        for ec in range(8):
                            nc.tensor.matmul(
                                pq,
                                wqk_sb[:, ec * 256 + m * 128 : ec * 256 + (m + 1) * 128],
                                xts[ec][:, n * 512 : (n + 1) * 512],
                                start=(ec == 0),
                                stop=(ec == 7),
                            )
                            yield 512 * PE_NS
                        nc.vector.tensor_scalar_add(
                            qkT_sb[:, m * S + n * 512 : m * S + (n + 1) * 512],
                            pq,
                            bqk_sb[:, m : m + 1],
                        )
                        yield 0.0
                    v_sb = v3t[b % 2]
                    vq = ps.tile([128, 512], f32, tag="acc", name="vq")
                    for sub in range(4):
                        c = n * 4 + sub
                        for ec in range(8):
                            nc.tensor.matmul(
                                vq[:, sub * 128 : (sub + 1) * 128],
                                xts[ec][:, c * 128 : (c + 1) * 128],
                                wv_sb[:, ec * 128 : (ec + 1) * 128],
                                start=(ec == 0),
                                stop=(ec == 7),
                            )
                            yield 128 * PE_NS
                    nc.vector.tensor_copy(
                        v_sb.rearrange("p (c h z) -> p c h z", c=NC, h=2)[
                            :, n * 4 : n * 4 + 4, :, 0:64
                        ],
                        vq.rearrange("p (c h dd) -> p c h dd", c=4, h=2),
                    )
                    yield 0.0

            def oproj_steps(b, pair, last=False):
                # o_proj: po[u, c] = sum_j pair[h][:, j*128:+128]^T wo_j
                # bias added on DVE during the PSUM->SBUF copy
                for h in range(2):
                    out_sb = sb.tile([128, E], f32, tag="osb", name="osb")
                    for n2 in range(2):
                        po = ps.tile([128, 512], f32, tag="acc", name="po")
                        for j in range(8):
                            nc.tensor.matmul(
                                po,
                                pair[h][:, j * 128 : (j + 1) * 128],
                                wo_sb[:, j * E + n2 * 512 : j * E + (n2 + 1) * 512],
                                start=(j == 0),
                                stop=(j == 7),
                            )
                            yield 512 * PE_NS
                        # all stores ride the SP ring: ACT ring config would
                        # stall the exp pacer; Pool ring SWDGE generation
                        # (~1us/store) would delay the diag mask-muls that
                        # gate PV
                        ring = nc.sync
                        if last and h == 1:
                            # final two chains: 256-col DVE+DMA pieces on
                            # alternating rings pipeline the kernel-exit tail
                            for z in range(2):
                                cl = n2 * 512 + z * 256
                                nc.vector.tensor_add(
                                    out_sb[:, cl : cl + 256],
                                    po[:, z * 256 : (z + 1) * 256],
                                    boeff_sb[:, h * E + cl : h * E + cl + 256],
                                )
                                rg = nc.sync if z == 0 else nc.scalar
                                rg.dma_start(
                                    out.ap()[b, h, :, cl : cl + 256],
                                    out_sb[:, cl : cl + 256],
                                )
                        else:
                            nc.vector.tensor_add(
                                out_sb[:, n2 * 512 : (n2 + 1) * 512],
                                po,
                                boeff_sb[:, h * E + n2 * 512 : h * E + (n2 + 1) * 512],
                            )
                            ring.dma_start(
                                out.ap()[b, h, :, n2 * 512 : (n2 + 1) * 512],
                                out_sb[:, n2 * 512 : (n2 + 1) * 512],
                            )
                        yield 0.0

            # ---- attention over 4 bands of 512 queries ----
            def emit_band2(qkT_sb, pair, g):
                # v2-style band for the LAST batch: PE-dominated per chunk
                # (no filler needed), 64-wide-ones rowsum replication, DVE
                # normalization scatter at band end. atts = the two banks of
                # one att2-tag tile.
                att2_t = ps.tile([128, 1024], f32, tag="att2", name="att2",
                                 bufs=1)
                atts = [att2_t[:, 0:512], att2_t[:, 512:1024]]
                nkj = 4 * g + 4

                def emit_pv2(kj, qo, ex):
                    for h in range(2):
                        nc.tensor.matmul(
                            atts[h][:, qo:512],
                            v3t[1][:, kj * 256 + h * 128 : kj * 256 + (h + 1) * 128],
                            ex[:, h * 512 + qo : (h + 1) * 512],
                            start=(kj == 0),
                            stop=(kj == nkj - 1),
                        )

                pend = []
                for kj in range(nkj):
                    qo = 128 * max(0, kj - 4 * g)
                    scp = ps.tile([128, 1024], f32, tag="sc", name="scp", bufs=2)
                    ex = sb3.tile([128, 1024], f16, tag="ex", name="ex", bufs=6)
                    for h in range(2):
                        nc.tensor.matmul(
                            scp[:, h * 512 + qo : (h + 1) * 512],
                            qkT_sb[h * 64 : (h + 1) * 64,
                                   S + kj * 128 : S + (kj + 1) * 128],
                            qkT_sb[h * 64 : (h + 1) * 64,
                                   g * 512 + qo : (g + 1) * 512],
                            start=True,
                            stop=True,
                            tile_position=(h * 64, 0),
                        )
                    nc.scalar.activation(
                        ex.rearrange("p (h q) -> p h q", h=2)[:, :, qo:512],
                        scp.rearrange("p (h q) -> p h q", h=2)[:, :, qo:512],
                        Exp,
                        scale=SCALE,
                    )
                    if kj >= 4 * g:  # diagonal chunk: zero q < k
                        nc.vector.tensor_mul(
                            ex.rearrange("p (h q) -> p h q", h=2)[
                                :, :, qo : qo + 128
                            ],
                            ex.rearrange("p (h q) -> p h q", h=2)[
                                :, :, qo : qo + 128
                            ],
                            trimask_sb.rearrange("p (h q) -> p h q", h=2),
                        )
                    # off-diagonal PV trails scores by one chunk (exp sem
                    # settled); diag immediate so the band end is not delayed
                    pend.append((kj, qo, ex))
                    if kj >= 4 * g - 1:
                        while pend:
                            emit_pv2(*pend.pop(0))
                    elif len(pend) > 1:
                        emit_pv2(*pend.pop(0))
                # normalize + scatter into o_proj pair layout
                for h in range(2):
                    rb = sb.tile([64, 512], f32, tag="rb", name="rb")
                    nc.vector.reciprocal(rb, atts[h][64:128, :])
                    attv = atts[h].rearrange(
                        "p (u2 w2 pr) -> p u2 w2 pr", u2=32, w2=8
                    )
                    rbv = rb.rearrange(
                        "p (u2 w2 pr) -> p u2 w2 pr", u2=32, w2=8
                    )
                    pav = pair[h].rearrange("p (j u) -> p u j", j=8)
                    for par in range(2):
                        nc.vector.tensor_mul(
                            pav[par * 64 : (par + 1) * 64,
                                g * 32 : (g + 1) * 32, :],
                            attv[0:64, :, :, par : par + 1],
                            rbv[0:64, :, :, par : par + 1],
                        )

            def emit_band(b, qkT_sb, pair, g, fill, fine_tail=False):
                # PSUM deps are tracked at BANK granularity: any read of an
                # accumulating bank serializes later matmul writes to it.
                # So: bank0 = h0 data [0:256) + 4 f16 transpose slots
                # [256:512); bank1 = h1 data [512:768) + all rowsums
                # [768:776). ALL normalization reads happen at band end,
                # after both banks' accumulations have stopped.
                v3_sb = v3t[b % 2]
                att2 = ps.tile([128, 1024], f32, tag="att2", name="att2", bufs=1)
                trv = att2[:, 256:512].bitcast(f16)   # [128, 512] f16

                def emit_pv(kj, lo, ex):
                    cost = 0.0
                    for h in range(2):
                        for qc in range(lo, 4):
                            exs = ex[:, h * 512 + qc * 128 : h * 512 + (qc + 1) * 128]
                            nc.tensor.matmul(
                                att2[:, h * 512 + qc * 64 : h * 512 + (qc + 1) * 64],
                                exs,
                                v3_sb[:, kj * 256 + h * 128 : kj * 256 + h * 128 + 64],
                                start=(kj == 0 and h == 0 and qc == 0),
                                stop=(kj == 4 * g + qc),
                                skip_group_check=True,
                            )
                            # softmax row-sum rides as a 1-column matmul of
                            # the ones column (same stationary ex chunk)
                            nc.tensor.matmul(
                                att2[:, 768 + h * 4 + qc : 768 + h * 4 + qc + 1],
                                exs,
                                v3_sb[:, kj * 256 + h * 128 + 64 : kj * 256 + h * 128 + 65],
                                start=(kj == 0 and h == 0 and qc == 0),
                                stop=(kj == 4 * g + qc),
                                skip_group_check=True,
                            )
                            cost += 65 * PE_NS
                    return cost

                nkj = 4 * g + 4
                pend = []
                for kj in range(nkj):
                    qo = 128 * max(0, kj - 4 * g)
                    lo = max(0, kj - 4 * g)
                    # slot order: PV/filler first, scores LAST — this gives
                    # the scp WAR (vs the exp two chunks back) extra slack
                    if len(pend) > 3:
                        fill.pe += emit_pv(*pend.pop(0))
                    fill.drain_until(fill.act)
                    scp = ps.tile([128, 1024], f32, tag="sc", name="scp", bufs=2)
                    ex = sb3.tile([128, 1024], f16, tag="ex", name="ex", bufs=6)
                    fill.pe += 2 * (512 - qo) * PE_NS
                    for h in range(2):
                        nc.tensor.matmul(
                            scp[:, h * 512 + qo : (h + 1) * 512],
                            qkT_sb[h * 64 : (h + 1) * 64,
                                   S + kj * 128 : S + (kj + 1) * 128],
                            qkT_sb[h * 64 : (h + 1) * 64,
                                   g * 512 + qo : (g + 1) * 512],
                            start=True,
                            stop=True,
                            tile_position=(h * 64, 0),
                        )
                    nc.scalar.activation(
                        ex.rearrange("p (h q) -> p h q", h=2)[:, :, qo:512],
                        scp.rearrange("p (h q) -> p h q", h=2)[:, :, qo:512],
                        Exp,
                        scale=SCALE,
                    )
                    fill.act += 2 * (512 - qo) * ACT_NS + 190
                    if kj >= 4 * g:  # diagonal chunk: zero q < k
                        # on the idle Pool engine: DVE is near-saturated
                        # during bands with norm/scatter/finalize work
                        nc.gpsimd.tensor_mul(
                            ex.rearrange("p (h q) -> p h q", h=2)[
                                :, :, qo : qo + 128
                            ],
                            ex.rearrange("p (h q) -> p h q", h=2)[
                                :, :, qo : qo + 128
                            ],
                            trimask_sb.rearrange("p (h q) -> p h q", h=2),
                        )
                    pend.append((kj, lo, ex))
                while pend:
                    fill.pe += emit_pv(*pend.pop(0))
                    fill.drain_min(400)
                # ---- band-end normalization block ----
                # one recip (all 8 rowsums), 8 muls, 4 transposes, 4
                # batched scatters; bank transitions are single-direction
                # (reads of data banks, then writes to bank0 tr slots)
                rcp = sb.tile([128, 8], f32, tag="rcp", name="rcp", bufs=2)
                nc.vector.reciprocal(rcp, att2[:, 768:776])
                nrms = []
                for qc in range(4):
                    nrm = sb.tile([128, 128], f16, tag="nrm", name="nrm",
                                  bufs=4)
                    for h in range(2):
                        nc.vector.tensor_scalar_mul(
                            nrm[:, h * 64 : (h + 1) * 64],
                            att2[:, h * 512 + qc * 64 : h * 512 + (qc + 1) * 64],
                            rcp[:, h * 4 + qc : h * 4 + qc + 1],
                        )
                    nrms.append(nrm)
                    fill.drain_min(350)
                fill.drain_min(600)
                for qc in range(4):
                    # start=False: land on the region zeroed by the band's
                    # PV start (HW zeroes the whole bank on start=True)
                    nc.tensor.matmul(
                        trv[:, qc * 128 : (qc + 1) * 128], nrms[qc], eye_sb,
                        is_transpose=True, start=False, stop=True,
                        skip_group_check=True,
                    )
                    fill.pe += 128 * PE_NS
                fill.drain_min(500)
                trb = trv.rearrange(
                    "p (qc ul j pr) -> p pr j qc ul", qc=4, ul=8, j=8
                )
                for h in range(2):
                    pav = pair[h].rearrange(
                        "p (j qcg ul) -> p j qcg ul", j=8, qcg=16
                    )
                    for par in range(2):
                        nc.vector.tensor_copy(
                            pav[par * 64 : (par + 1) * 64, :,
                                g * 4 : g * 4 + 4, :],
                            trb[h * 64 : (h + 1) * 64, par],
                        )
                # guard reads: the next band's start=True PVs zero their
                # whole bank (invisible to the range tracker); these trailing
                # DVE reads of cols 0 and 768 give those PVs a WAR dep that
                # orders the zeroing after this band's scatters/muls
                grd = sb.tile([128, 2], f32, tag="grd", name="grd", bufs=2)
                nc.vector.tensor_copy(grd[:, 0:1], att2[:, 0:1])
                nc.vector.tensor_copy(grd[:, 1:2], att2[:, 768:769])
                fill.drain_min(400)

            # ================= prologue ==================================
            # SP DMA queue order is the schedule —
            # wqk | xt(0) fine | wv | xt(1) | wo+boeff | xt(2) ...
            # small consts ride the Pool/SWDGE path, off the HWDGE queue
            nc.gpsimd.dma_start(bqk_sb, bqk.ap())
            nc.gpsimd.dma_start(trimask_sb, trimask2.ap())
            nc.gpsimd.dma_start(eye_sb, eye.ap())
            xts0 = emit_xt_dma(0, fine=True, wqk_rest=(wqk_sb, wqk_dram))
            xtss = {}
            nc.sync.dma_start(
                wv_sb.rearrange("p (ec c) -> p ec c", ec=8),
                wv.ap().rearrange("(ec p) c -> p ec c", p=128),
            )
            qkts = {0: emit_proj0(0, xts0)}
            xtss[1] = emit_xt_dma(1)
            nc.sync.dma_start(
                wo_sb.rearrange("p (j c) -> p j c", j=8),
                wo.ap().rearrange("(j p) c -> p j c", p=128),
            )
            nc.sync.dma_start(boeff_sb, boeff.ap())

            fill = Filler()
            # n=2,3 chains of proj(b) are deferred INTO bands(b): band g
            # only reads qkT/v from chains n<=g. proj(3) fully drains during
            # bands(2) because the v2-style bands(3) consume no filler.
            defer = proj_steps(0, xts0, qkts[0], ns=(2, 3))
            pairs = {}
            for b in range(3):
                pairs[b] = [
                    sb.tile([128, 8 * 128], f16, tag=f"pair{h}", name=f"pair{h}",
                            bufs=4)
                    for h in range(2)
                ]
                if defer is not None:
                    fill.add(defer)
                qkts[b + 1] = sb.tile([128, 2 * S], f16, tag="qkT", name="qkT")
                xts_n = xtss.pop(b + 1)
                if b < 2:
                    proj_gen = proj_steps(b + 1, xts_n, qkts[b + 1], ns=(0, 1))
                    defer_next = proj_steps(b + 1, xts_n, qkts[b + 1],
                                            ns=(2, 3))
                else:
                    proj_gen = proj_steps(3, xts_n, qkts[3], ns=range(4))
                    defer_next = None
                fill.add(proj_gen)
                for g in range(4):
                    if g == 1 and defer is not None:
                        # deadline: band 2 reads qkT/v from n=2; one band
                        # early so the v-copy DVE latency is hidden
                        fill.drain_gen(defer)
                    emit_band(b, qkts[b], pairs[b], g, fill)
                # hard deadline: proj(b+1) emitted before bands(b+1)
                fill.drain_gen(proj_gen)
                fill.add(oproj_steps(b, pairs[b]), proj=False)
                defer = defer_next
                del qkts[b]
                if b + 2 < B:
                    xtss[b + 2] = emit_xt_dma(b + 2)
            # last batch: v2-style self-filling bands (PE-dominated chunks)
            pairs[3] = [
                sb.tile([128, 8 * 128], f16, tag=f"pair{h}", name=f"pair{h}",
                        bufs=4)
                for h in range(2)
            ]
            for g in range(4):
                # ~1.2us of oproj filler covers the 2-chunk exp pipeline
                # fill at each band start
                fill.drain_until(fill.pe + 1200)
                emit_band2(qkts[3], pairs[3], g)
            # remaining o_proj emitted LAST as PE filler-of-last-resort
            fill.drain_all()
            for _ in oproj_steps(3, pairs[3], last=True):
                pass

    nc.compile()
    return nc


def _get_program():
    if "nc" not in _CACHE:
        _CACHE["nc"] = _build_program()
    return _CACHE["nc"]


def _host_inputs(x, Wqkv, bqkv, Wo, bo):
    """Per-core input maps (host-side layout prep: cast/slice/fold)."""
    xT = np.ascontiguousarray(x.transpose(0, 2, 1)).astype(np.float16)

    wo16 = Wo.astype(np.float16)

    # fold v-bias through attention (softmax rows sum to 1) into o_proj bias:
    # boeff_h = bo + bv_h @ sum_w Wo[w*64+d, :]
    wsum = Wo.reshape(16, 64, E).sum(axis=0)      # [64, E] float32

    k_idx = np.arange(128)[:, None]
    q_idx = np.arange(128)[None, :]
    tri = (k_idx <= q_idx).astype(np.float16)
    trimask2 = np.concatenate([tri, tri], axis=1)  # [128, 256]
    eye = np.eye(128, dtype=np.float16)

    in_maps = []
    for c in range(NCORES):
        h0, h1 = HPC * c, HPC * c + 1
        qcols = list(range(h0 * 3 * D, h0 * 3 * D + 64)) + list(
            range(h1 * 3 * D, h1 * 3 * D + 64)
        )
        kcols = [cc + 64 for cc in qcols]
        vcols = [cc + 128 for cc in qcols]
        bqk_arr = np.stack(
            [bqkv[qcols].astype(np.float32), bqkv[kcols].astype(np.float32)], axis=1
        )  # [128, 2]
        boeff = np.zeros((128, 2 * E), np.float32)
        for i, h in enumerate((h0, h1)):
            bv = bqkv[h * 3 * D + 128 : h * 3 * D + 192].astype(np.float32)
            boeff[:, i * E : (i + 1) * E] = (bo.astype(np.float32) + bv @ wsum)[None, :]
        in_maps.append(
            {
                "xT": xT,
                "wqk": np.ascontiguousarray(Wqkv[:, qcols + kcols]).astype(np.float16),
                "wv": np.ascontiguousarray(Wqkv[:, vcols]).astype(np.float16),
                "bqk": np.ascontiguousarray(bqk_arr),
                "wo": wo16,
                "boeff": boeff,
                "trimask2": trimask2,
                "eye": eye,
            }
        )
    return in_maps


def kernel(x, mask, Wqkv, bqkv, Wo, bo, _n_cores=NCORES, _trace=False):
    """Full-input, full-output MHA. `mask` is the causal tril mask (hardcoded)."""
    from concourse.bass_utils import run_bass_kernel_spmd

    nc = _get_program()
    in_maps = _host_inputs(
        np.asarray(x), np.asarray(Wqkv), np.asarray(bqkv), np.asarray(Wo), np.asarray(bo)
    )[:_n_cores]
    res = run_bass_kernel_spmd(
        nc, in_maps, core_ids=list(range(_n_cores)), trace=_trace
    )
    out_full = np.zeros((B, S, E), np.float32)
    for c in range(_n_cores):
        o = res.results[c]["out"]  # [B, HPC, 128, E]
        for h in range(HPC):
            g = HPC * c + h
            out_full[:, g * 128 : (g + 1) * 128, :] = o[:, h]
    _CACHE["last_results"] = res
    return out_full


# revision 57
# speedup vs baseline: 1.0374x; 1.0124x over previous
"""Trainium2 Bass kernel for nn_MultiHeadAttention_46093589021200.

Causal MHA: B=4, S=2048, E=1024, H=16, D=64, with the reference's
"no-transpose-back" reshape (b,h,s,d)->(b,s,e) before the output projection.

Sharding: pure head-parallel, 2 heads per core, zero collectives.
Because of the reshape quirk, output rows s' in [h*128,(h+1)*128) depend only
on head h, so each core produces two independent 128-row output bands per
batch.

v3 design notes (vs v2; 241624ns baseline):
  - PV restructured to put q on PSUM partitions: att2[q,65] accumulates
    exT_chunk[k,q] @ [v_h|ones][k,65] over k-chunks. Cost model charges
    N(=65) per matmul instead of the q-band width (<=512), cutting PV from
    17408 to 8840 cycles per head-batch. The single ones column accumulates
    the softmax row-sum (replacing v2's 64-wide ones block).
  - softmax normalization becomes a per-partition scalar op: reciprocal of
    att2[:,64] then tensor_scalar_mul -> norm [q, (h,d)] f16 in SBUF.
  - o_proj needs (w,d)-on-partitions, so one PE transpose (via identity
    matmul) per 128-q-chunk converts norm [q,128] -> [128,q] f16 in a
    bitcast region of the att2 PSUM tile (banks are exactly full: 2x sc
    [128,1024] + att2 [128,1024] + 2x acc [128,512] = 8 banks). DVE then
    scatters into the o_proj "pair" layout (partition (w%2)*64+d, col
    (w//2)*128+u, with q = u*16+w).
  - with PV halved, attention bands are locally ACT(exp)-bound, and the PE
    executes in order -- so next-batch projection chains and o_proj(b-1) are
    interleaved at CHUNK granularity via a filler-generator queue (v2's
    en-bloc emission after bands would stall behind exp sems). PV for chunk
    kj is emitted after scores(kj+1) so the exp(kj) sem has settled.
  - per-core PE cycles: qk 131072 + v 65536 + scores 139264 + pv 70720 +
    transpose 8192 + o_proj 65536 = 480320 (~200.2us at 2.4GHz).

HW rules learned by probing (CoreSim accepts all of these, HW does not):
  - matmuls from DIFFERENT PE row groups must not write the same PSUM bank
    (same-row-group region-sharing of a bank is fine).
  - column-positioned matmuls (tile_position=(0,32j), PSUM output at a
    partition offset) mis-execute. Matmul lhsT/rhs share their SBUF base
    partition. DVE ops MAY write partition-shifted outputs.
"""

import sys

if "/opt/trn_rl_repo" not in sys.path:
    sys.path.insert(0, "/opt/trn_rl_repo")

import numpy as np

B, S, E, H = 4, 2048, 1024, 16
D = E // H          # 64
NCORES = 8
HPC = H // NCORES   # heads per core = 2
SCALE = 1.0 / float(np.sqrt(D))
NC = S // 128       # 16 key chunks

_CACHE = {}

PE_NS = 1.0 / 2.4   # ns per streamed output column at full pstate
ACT_NS = 1.0 / 1.2  # ns per lane-element on the activation engine


class Filler:
    """Two-priority queue of emission generators, drained in cost-budgeted
    slices.

    Generators emit instructions between yields; each yield value is the
    PE-cost (ns) of what was just emitted. Band emission pulls from this
    queue to keep the in-order PE stream fed through ACT-bound stretches.
    proj generators (hard emission deadline: before the next batch's bands)
    drain first; oproj generators are deliberately held back so the final
    batch -- which has no next-batch projection -- still has PE filler.
    """

    def __init__(self):
        self.projq = []
        self.oprojq = []
        self.pe = 0.0    # cumulative PE ns emitted (bands + fillers)
        self.act = 0.0   # cumulative ACT ns emitted

    def add(self, gen, proj=True):
        (self.projq if proj else self.oprojq).append(gen)

    def drain_until(self, target):
        """Pull filler until cumulative emitted PE work reaches target."""
        while self.pe < target:
            q = self.projq if self.projq else self.oprojq
            if not q:
                return
            try:
                self.pe += max(next(q[0]), 1.0)
            except StopIteration:
                q.pop(0)

    def drain_min(self, ns):
        """Pull ~ns of filler, but never run the emitted-PE clock more than
        ~1.2us ahead of the ACT clock (over-pulling here exhausts the oproj
        reserve before the last batch needs it)."""
        self.drain_until(min(self.pe + ns, self.act + 1200))

    def drain_gen(self, gen):
        """Force-finish one generator (emission-order deadline)."""
        if gen in self.projq:
            self.projq.remove(gen)
        for c in gen:
            self.pe += max(c, 1.0)

    def drain_all(self):
        for q in (self.projq, self.oprojq):
            while q:
                g = q.pop(0)
                for c in g:
                    self.pe += max(c, 1.0)


def _build_program():
    import concourse.bass as bass  # noqa: F401
    import concourse.tile as tile
    from concourse import bacc, mybir

    f16 = mybir.dt.float16
    f32 = mybir.dt.float32
    Exp = mybir.ActivationFunctionType.Exp

    nc = bacc.Bacc("TRN2", target_bir_lowering=False, debug=False)

    xT = nc.dram_tensor("xT", [B, E, S], f16, kind="ExternalInput")
    wqk = nc.dram_tensor("wqk", [E, 256], f16, kind="ExternalInput")
    wv = nc.dram_tensor("wv", [E, 128], f16, kind="ExternalInput")
    bqk = nc.dram_tensor("bqk", [128, 2], f32, kind="ExternalInput")
    wo = nc.dram_tensor("wo", [E, E], f16, kind="ExternalInput")
    boeff = nc.dram_tensor("boeff", [128, 2 * E], f32, kind="ExternalInput")
    trimask2 = nc.dram_tensor("trimask2", [128, 256], f16, kind="ExternalInput")
    eye = nc.dram_tensor("eye", [128, 128], f16, kind="ExternalInput")
    out = nc.dram_tensor("out", [B, HPC, 128, E], f32, kind="ExternalOutput")

    with tile.TileContext(nc) as tc:
        with (
            tc.tile_pool(name="const", bufs=1) as cp,
            tc.tile_pool(name="sb", bufs=2) as sb,
            tc.tile_pool(name="sb3", bufs=3) as sb3,
            tc.tile_pool(name="ps", bufs=2, space="PSUM") as ps,
        ):
            # ---- constants resident in SBUF for the whole kernel ----
            # ec0 slice first (364ns) so the first matmul unblocks early;
            # the remainder queues behind the first xt piece
            wqk_sb = cp.tile([128, 8 * 256], f16)     # [p, ec*256 + col]
            wqk_dram = wqk.ap().rearrange("(ec p) c -> p ec c", p=128)
            nc.sync.dma_start(wqk_sb[:, 0:256], wqk_dram[:, 0])
            # wv/bqk/trimask DMAs are deferred until after xt(0) (see below)
            wv_sb = cp.tile([128, 8 * 128], f16)      # [p, ec*128 + vcol]
            bqk_sb = cp.tile([128, 2], f32)
            trimask_sb = cp.tile([128, 256], f16)
            eye_sb = cp.tile([128, 128], f16)
            # o_proj constants allocated here, DMA'd after the prologue so
            # they don't delay xt(0) on the shared DMA engines
            wo_sb = cp.tile([128, 8 * E], f16)        # [p, j*1024 + c]
            boeff_sb = cp.tile([128, 2 * E], f32)     # [p, h*1024 + c], bcast rows

            # persistent double-buffered v tiles: per s-chunk c the 130-col
            # group [v_h0(64) | one | v_h1(64) | one]; the single ones column
            # accumulates the softmax row-sum during PV.
            # v2-style 256-stride v_aug tiles [v_h0|ones(64)|v_h1|ones(64)]:
            # 256-aligned moving-operand offsets (the earlier 130-stride
            # layout put PV rhs at odd 130-byte offsets, which mis-executes
            # on HW), ones col 64 doubles as the v3-band rowsum column and
            # the 64-wide block serves the v2-style last-batch band.
            v3t = [cp.tile([128, NC * 256], f16, name=f"v3_{i}") for i in range(2)]
            for t in v3t:
                tv = t.rearrange("p (c h z) -> p c h z", c=NC, h=2)
                nc.gpsimd.memset(tv[:, :, :, 64:128], 1.0)

            def emit_xt_dma(b, fine=False, wqk_rest=None):
                # 8 per-ec tiles so projection chains start as each DMA lands.
                # fine=True (first batch): split per (n-block, ec) so the
                # prologue chains complete incrementally from ~3us.
                xts = [
                    sb.tile([128, S], f16, tag=f"xt{ec}", name=f"xt{ec}", bufs=3)
                    for ec in range(8)
                ]
                xt_dram = xT.ap()[b].rearrange("(ec p) s -> p ec s", p=128)
                if fine:
                    # halves: 728ns transfer > 625ns HWDGE overhead, so the
                    # piece stream stays transfer-bound. The very first piece
                    # is a 512-col quarter: exactly what the first projection
                    # matmul needs, so it unblocks ~360ns earlier.
                    for n in range(2):
                        for ec in range(8):
                            if ec == 0 and n == 0:
                                nc.sync.dma_start(
                                    xts[0][:, 0:512], xt_dram[:, 0, 0:512]
                                )
                                nc.sync.dma_start(
                                    xts[0][:, 512:1024], xt_dram[:, 0, 512:1024]
                                )
                            else:
                                nc.sync.dma_start(
                                    xts[ec][:, n * 1024 : (n + 1) * 1024],
                                    xt_dram[:, ec, n * 1024 : (n + 1) * 1024],
                                )
                            if wqk_rest is not None and ec == 0 and n == 0:
                                wsb, wdr = wqk_rest
                                nc.sync.dma_start(
                                    wsb.rearrange("p (ec c) -> p ec c", ec=8)[
                                        :, 1:8
                                    ],
                                    wdr[:, 1:8],
                                )
                else:
                    for ec in range(8):
                        nc.sync.dma_start(xts[ec], xt_dram[:, ec])
                return xts

            def emit_v_group(b, xts, sc4, tag="acc", bufs=2):
                # v in natural [s, d] layout, 4 s-chunks per PSUM bank
                v3_sb = v3t[b % 2]
                vq = ps.tile([128, 512], f32, tag=tag, name="vq", bufs=bufs)
                for sub in range(4):
                    c = sc4 * 4 + sub
                    for ec in range(8):
                        nc.tensor.matmul(
                            vq[:, sub * 128 : (sub + 1) * 128],
                            xts[ec][:, c * 128 : (c + 1) * 128],
                            wv_sb[:, ec * 128 : (ec + 1) * 128],
                            start=(ec == 0),
                            stop=(ec == 7),
                        )
                # copy into v3 chunks (skips the ones columns)
                nc.vector.tensor_copy(
                    v3_sb.rearrange("p (c h z) -> p c h z", c=NC, h=2)[
                        :, sc4 * 4 : sc4 * 4 + 4, :, 0:64
                    ],
                    vq.rearrange("p (c h dd) -> p c h dd", c=4, h=2),
                )

            def emit_proj0(b, xts):
                # prologue-only en-bloc projection (DMA-paced anyway):
                # q/k chains then v groups, borrowing the still-idle PSUM
                # tags so several chains progress as the fine pieces land.
                # only n=0,1 here: the n=2,3 chains are deferred into the
                # filler queue as READY work for bands(0), whose natural
                # filler (proj(1)) is blocked on the xt(1) DMA until ~26us.
                tags = [("att2", 1), ("sc", 2), ("acc", 2)]
                qkT_sb = sb.tile([128, 2 * S], f16, tag="qkT", name="qkT")
                for n in range(2):
                    for m in range(2):
                        tg, bf = tags[(n * 2 + m) % 3]
                        pq = ps.tile([128, 512], f32, tag=tg, name="pq", bufs=bf)
                        for ec in range(8):
                            nc.tensor.matmul(
                                pq,
                                wqk_sb[:, ec * 256 + m * 128 : ec * 256 + (m + 1) * 128],
                                xts[ec][:, n * 512 : (n + 1) * 512],
                                start=(ec == 0),
                                stop=(ec == 7),
                            )
                        nc.vector.tensor_scalar_add(
                            qkT_sb[:, m * S + n * 512 : m * S + (n + 1) * 512],
                            pq,
                            bqk_sb[:, m : m + 1],
                        )
                vtags = [("sc", 2), ("att2", 1)]
                for sc4 in range(2):
                    tg, bf = vtags[sc4]
                    emit_v_group(b, xts, sc4, tag=tg, bufs=bf)
                return qkT_sb

            def proj_steps(b, xts, qkT_sb, ns=range(4)):
                # filler generator: q/k chain pair then v group, per n-block
                # (matches the order bands(b) consume them: band g needs
                # qk n<=g, k all ... emitted ascending; v groups 0..g).
                for n in ns:
                    for m in range(2):
                        pq = ps.tile([128, 512], f32, tag="acc", name="pq")
                        for ec in range(8):
                            nc.tensor.matmul(
                                pq,
                                wqk_sb[:, ec * 256 + m * 128 : ec * 256 + (m + 1) * 128],
                                xts[ec][:, n * 512 : (n + 1) * 512],
                                start=(ec == 0),
                                stop=(ec == 7),
                            )
                            yield 512 * PE_NS
                        nc.vector.tensor_scalar_add(
                            qkT_sb[:, m * S + n * 512 : m * S + (n + 1) * 512],
                            pq,
                            bqk_sb[:, m : m + 1],
                        )
                        yield 0.0
                    v_sb = v3t[b % 2]
                    vq = ps.tile([128, 512], f32, tag="acc", name="vq")
                    for sub in range(4):
                        c = n * 4 + sub
                        for ec in range(8):
                            nc.tensor.matmul(
                                vq[:, sub * 128 : (sub + 1) * 128],
                                xts[ec][:, c * 128 : (c + 1) * 128],
                                wv_sb[:, ec * 128 : (ec + 1) * 128],
                                start=(ec == 0),
                                stop=(ec == 7),
                            )
                            yield 128 * PE_NS
                    nc.vector.tensor_copy(
                        v_sb.rearrange("p (c h z) -> p c h z", c=NC, h=2)[
                            :, n * 4 : n * 4 + 4, :, 0:64
                        ],
                        vq.rearrange("p (c h dd) -> p c h dd", c=4, h=2),
                    )
                    yield 0.0

            def oproj_steps(b, pair, last=False):
                # o_proj: po[u, c] = sum_j pair[h][:, j*128:+128]^T wo_j
                # bias added on DVE during the PSUM->SBUF copy
                for h in range(2):
                    out_sb = sb.tile([128, E], f32, tag="osb", name="osb")
                    for n2 in range(2):
                        po = ps.tile([128, 512], f32, tag="acc", name="po")
                        for j in range(8):
                            nc.tensor.matmul(
                                po,
                                pair[h][:, j * 128 : (j + 1) * 128],
                                wo_sb[:, j * E + n2 * 512 : j * E + (n2 + 1) * 512],
                                start=(j == 0),
                                stop=(j == 7),
                            )
                            yield 512 * PE_NS
                        # all stores ride the SP ring: ACT ring config would
                        # stall the exp pacer; Pool ring SWDGE generation
                        # (~1us/store) would delay the diag mask-muls that
                        # gate PV
                        ring = nc.sync
                        if last and h == 1:
                            # final two chains: 256-col DVE+DMA pieces on
                            # alternating rings pipeline the kernel-exit tail
                            for z in range(2):
                                cl = n2 * 512 + z * 256
                                nc.vector.tensor_add(
                                    out_sb[:, cl : cl + 256],
                                    po[:, z * 256 : (z + 1) * 256],
                                    boeff_sb[:, h * E + cl : h * E + cl + 256],
                                )
                                rg = nc.sync if z == 0 else nc.scalar
                                rg.dma_start(
                                    out.ap()[b, h, :, cl : cl + 256],
                                    out_sb[:, cl : cl + 256],
                                )
                        else:
                            nc.vector.tensor_add(
                                out_sb[:, n2 * 512 : (n2 + 1) * 512],
                                po,
                                boeff_sb[:, h * E + n2 * 512 : h * E + (n2 + 1) * 512],
                            )
                            ring.dma_start(
                                out.ap()[b, h, :, n2 * 512 : (n2 + 1) * 512],
                                out_sb[:, n2 * 512 : (n2 + 1) * 512],
                            )
                        yield 0.0

            # ---- attention over 4 bands of 512 queries ----
            def emit_band2(qkT_sb, pair, g):
                # v2-style band for the LAST batch: PE-dominated per chunk
                # (no filler needed), 64-wide-ones rowsum replication, DVE
                # normalization scatter at band end. atts = the two banks of
                # one att2-tag tile.
                att2_t = ps.tile([128, 1024], f32, tag="att2", name="att2",
                                 bufs=1)
                atts = [att2_t[:, 0:512], att2_t[:, 512:1024]]
                nkj = 4 * g + 4

                def emit_pv2(kj, qo, ex):
                    for h in range(2):
                        nc.tensor.matmul(
                            atts[h][:, qo:512],
                            v3t[1][:, kj * 256 + h * 128 : kj * 256 + (h + 1) * 128],
                            ex[:, h * 512 + qo : (h + 1) * 512],
                            start=(kj == 0),
                            stop=(kj == nkj - 1),
                        )

                pend = []
                for kj in range(nkj):
                    qo = 128 * max(0, kj - 4 * g)
                    scp = ps.tile([128, 1024], f32, tag="sc", name="scp", bufs=2)
                    ex = sb3.tile([128, 1024], f16, tag="ex", name="ex", bufs=8)
                    for h in range(2):
                        nc.tensor.matmul(
                            scp[:, h * 512 + qo : (h + 1) * 512],
                            qkT_sb[h * 64 : (h + 1) * 64,
                                   S + kj * 128 : S + (kj + 1) * 128],
                            qkT_sb[h * 64 : (h + 1) * 64,
                                   g * 512 + qo : (g + 1) * 512],
                            start=True,
                            stop=True,
                            tile_position=(h * 64, 0),
                        )
                    nc.scalar.activation(
                        ex.rearrange("p (h q) -> p h q", h=2)[:, :, qo:512],
                        scp.rearrange("p (h q) -> p h q", h=2)[:, :, qo:512],
                        Exp,
                        scale=SCALE,
                    )
                    if kj >= 4 * g:  # diagonal chunk: zero q < k
                        nc.vector.tensor_mul(
                            ex.rearrange("p (h q) -> p h q", h=2)[
                                :, :, qo : qo + 128
                            ],
                            ex.rearrange("p (h q) -> p h q", h=2)[
                                :, :, qo : qo + 128
                            ],
                            trimask_sb.rearrange("p (h q) -> p h q", h=2),
                        )
                    # off-diagonal PV trails scores by one chunk (exp sem
                    # settled); diag immediate so the band end is not delayed
                    pend.append((kj, qo, ex))
                    if kj >= 4 * g - 1:
                        while pend:
                            emit_pv2(*pend.pop(0))
                    elif len(pend) > 1:
                        emit_pv2(*pend.pop(0))
                # normalize + scatter into o_proj pair layout
                for h in range(2):
                    rb = sb.tile([64, 512], f32, tag="rb", name="rb")
                    nc.vector.reciprocal(rb, atts[h][64:128, :])
                    attv = atts[h].rearrange(
                        "p (u2 w2 pr) -> p u2 w2 pr", u2=32, w2=8
                    )
                    rbv = rb.rearrange(
                        "p (u2 w2 pr) -> p u2 w2 pr", u2=32, w2=8
                    )
                    pav = pair[h].rearrange("p (j u) -> p u j", j=8)
                    for par in range(2):
                        nc.vector.tensor_mul(
                            pav[par * 64 : (par + 1) * 64,
                                g * 32 : (g + 1) * 32, :],
                            attv[0:64, :, :, par : par + 1],
                            rbv[0:64, :, :, par : par + 1],
                        )

            def emit_band(b, qkT_sb, pair, g, fill, fine_tail=False):
                # PSUM deps are tracked at BANK granularity: any read of an
                # accumulating bank serializes later matmul writes to it.
                # So: bank0 = h0 data [0:256) + 4 f16 transpose slots
                # [256:512); bank1 = h1 data [512:768) + all rowsums
                # [768:776). ALL normalization reads happen at band end,
                # after both banks' accumulations have stopped.
                v3_sb = v3t[b % 2]
                att2 = ps.tile([128, 1024], f32, tag="att2", name="att2", bufs=1)
                trv = att2[:, 256:512].bitcast(f16)   # [128, 512] f16

                def emit_pv(kj, lo, ex):
                    cost = 0.0
                    for h in range(2):
                        for qc in range(lo, 4):
                            exs = ex[:, h * 512 + qc * 128 : h * 512 + (qc + 1) * 128]
                            nc.tensor.matmul(
                                att2[:, h * 512 + qc * 64 : h * 512 + (qc + 1) * 64],
                                exs,
                                v3_sb[:, kj * 256 + h * 128 : kj * 256 + h * 128 + 64],
                                start=(kj == 0 and h == 0 and qc == 0),
                                stop=(kj == 4 * g + qc),
                                skip_group_check=True,
                            )
                            # softmax row-sum rides as a 1-column matmul of
                            # the ones column (same stationary ex chunk)
                            nc.tensor.matmul(
                                att2[:, 768 + h * 4 + qc : 768 + h * 4 + qc + 1],
                                exs,
                                v3_sb[:, kj * 256 + h * 128 + 64 : kj * 256 + h * 128 + 65],
                                start=(kj == 0 and h == 0 and qc == 0),
                                stop=(kj == 4 * g + qc),
                                skip_group_check=True,
                            )
                            cost += 65 * PE_NS
                    return cost

                nkj = 4 * g + 4
                pend = []
                for kj in range(nkj):
                    qo = 128 * max(0, kj - 4 * g)
                    lo = max(0, kj - 4 * g)
                    # slot order: PV/filler first, scores LAST — this gives
                    # the scp WAR (vs the exp two chunks back) extra slack
                    if len(pend) > 5:
                        fill.pe += emit_pv(*pend.pop(0))
                    fill.drain_until(fill.act)
                    scp = ps.tile([128, 1024], f32, tag="sc", name="scp", bufs=2)
                    ex = sb3.tile([128, 1024], f16, tag="ex", name="ex", bufs=8)
                    fill.pe += 2 * (512 - qo) * PE_NS
                    for h in range(2):
                        nc.tensor.matmul(
                            scp[:, h * 512 + qo : (h + 1) * 512],
                            qkT_sb[h * 64 : (h + 1) * 64,
                                   S + kj * 128 : S + (kj + 1) * 128],
                            qkT_sb[h * 64 : (h + 1) * 64,
                                   g * 512 + qo : (g + 1) * 512],
                            start=True,
                            stop=True,
                            tile_position=(h * 64, 0),
                        )
                    nc.scalar.activation(
                        ex.rearrange("p (h q) -> p h q", h=2)[:, :, qo:512],
                        scp.rearrange("p (h q) -> p h q", h=2)[:, :, qo:512],
                        Exp,
                        scale=SCALE,
                    )
                    fill.act += 2 * (512 - qo) * ACT_NS + 190
                    if kj >= 4 * g:  # diagonal chunk: zero q < k
                        # on the idle Pool engine: DVE is near-saturated
                        # during bands with norm/scatter/finalize work
                        nc.gpsimd.tensor_mul(
                            ex.rearrange("p (h q) -> p h q", h=2)[
                                :, :, qo : qo + 128
                            ],
                            ex.rearrange("p (h q) -> p h q", h=2)[
                                :, :, qo : qo + 128
                            ],
                            trimask_sb.rearrange("p (h q) -> p h q", h=2),
                        )
                    pend.append((kj, lo, ex))
                while pend:
                    fill.pe += emit_pv(*pend.pop(0))
                    fill.drain_min(400)
                # ---- band-end normalization block ----
                # one recip (all 8 rowsums), 8 muls, 4 transposes, 4
                # batched scatters; bank transitions are single-direction
                # (reads of data banks, then writes to bank0 tr slots)
                rcp = sb.tile([128, 8], f32, tag="rcp", name="rcp", bufs=2)
                nc.vector.reciprocal(rcp, att2[:, 768:776])
                nrms = []
                for qc in range(4):
                    nrm = sb.tile([128, 128], f16, tag="nrm", name="nrm",
                                  bufs=4)
                    for h in range(2):
                        nc.vector.tensor_scalar_mul(
                            nrm[:, h * 64 : (h + 1) * 64],
                            att2[:, h * 512 + qc * 64 : h * 512 + (qc + 1) * 64],
                            rcp[:, h * 4 + qc : h * 4 + qc + 1],
                        )
                    nrms.append(nrm)
                    fill.drain_min(350)
                fill.drain_min(600)
                for qc in range(4):
                    # start=False: land on the region zeroed by the band's
                    # PV start (HW zeroes the whole bank on start=True)
                    nc.tensor.matmul(
                        trv[:, qc * 128 : (qc + 1) * 128], nrms[qc], eye_sb,
                        is_transpose=True, start=False, stop=True,
                        skip_group_check=True,
                    )
                    fill.pe += 128 * PE_NS
                fill.drain_min(500)
                trb = trv.rearrange(
                    "p (qc ul j pr) -> p pr j qc ul", qc=4, ul=8, j=8
                )
                for h in range(2):
                    pav = pair[h].rearrange(
                        "p (j qcg ul) -> p j qcg ul", j=8, qcg=16
                    )
                    for par in range(2):
                        nc.vector.tensor_copy(
                            pav[par * 64 : (par + 1) * 64, :,
                                g * 4 : g * 4 + 4, :],
                            trb[h * 64 : (h + 1) * 64, par],
                        )
                # guard reads: the next band's start=True PVs zero their
                # whole bank (invisible to the range tracker); these trailing
                # DVE reads of cols 0 and 768 give those PVs a WAR dep that
                # orders the zeroing after this band's scatters/muls
                grd = sb.tile([128, 2], f32, tag="grd", name="grd", bufs=2)
                nc.vector.tensor_copy(grd[:, 0:1], att2[:, 0:1])
                nc.vector.tensor_copy(grd[:, 1:2], att2[:, 768:769])
                fill.drain_min(400)

            # ================= prologue ==================================
            # SP DMA queue order is the schedule —
            # wqk | xt(0) fine | wv | xt(1) | wo+boeff | xt(2) ...
            # small consts ride the Pool/SWDGE path, off the HWDGE queue
            nc.gpsimd.dma_start(bqk_sb, bqk.ap())
            nc.gpsimd.dma_start(trimask_sb, trimask2.ap())
            nc.gpsimd.dma_start(eye_sb, eye.ap())
            xts0 = emit_xt_dma(0, fine=True, wqk_rest=(wqk_sb, wqk_dram))
            xtss = {}
            nc.sync.dma_start(
                wv_sb.rearrange("p (ec c) -> p ec c", ec=8),
                wv.ap().rearrange("(ec p) c -> p ec c", p=128),
            )
            qkts = {0: emit_proj0(0, xts0)}
            xtss[1] = emit_xt_dma(1)
            nc.sync.dma_start(
                wo_sb.rearrange("p (j c) -> p j c", j=8),
                wo.ap().rearrange("(j p) c -> p j c", p=128),
            )
            nc.sync.dma_start(boeff_sb, boeff.ap())

            fill = Filler()
            # n=2,3 chains of proj(b) are deferred INTO bands(b): band g
            # only reads qkT/v from chains n<=g. proj(3) fully drains during
            # bands(2) because the v2-style bands(3) consume no filler.
            defer = proj_steps(0, xts0, qkts[0], ns=(2, 3))
            pairs = {}
            for b in range(3):
                pairs[b] = [
                    sb.tile([128, 8 * 128], f16, tag=f"pair{h}", name=f"pair{h}",
                            bufs=4)
                    for h in range(2)
                ]
                if defer is not None:
                    fill.add(defer)
                qkts[b + 1] = sb.tile([128, 2 * S], f16, tag="qkT", name="qkT")
                xts_n = xtss.pop(b + 1)
                if b < 2:
                    proj_gen = proj_steps(b + 1, xts_n, qkts[b + 1], ns=(0, 1))
                    defer_next = proj_steps(b + 1, xts_n, qkts[b + 1],
                                            ns=(2, 3))
                else:
                    proj_gen = proj_steps(3, xts_n, qkts[3], ns=range(4))
                    defer_next = None
                fill.add(proj_gen)
                for g in range(4):
                    if g == 1 and defer is not None:
                        # deadline: band 2 reads qkT/v from n=2; one band
                        # early so the v-copy DVE latency is hidden
                        fill.drain_gen(defer)
                    emit_band(b, qkts[b], pairs[b], g, fill)
                # hard deadline: proj(b+1) emitted before bands(b+1)
                fill.drain_gen(proj_gen)
                fill.add(oproj_steps(b, pairs[b]), proj=False)
                defer = defer_next
                del qkts[b]
                if b + 2 < B:
                    xtss[b + 2] = emit_xt_dma(b + 2)
            # last batch: v2-style self-filling bands (PE-dominated chunks)
            pairs[3] = [
                sb.tile([128, 8 * 128], f16, tag=f"pair{h}", name=f"pair{h}",
                        bufs=4)
                for h in range(2)
            ]
            for g in range(4):
                # ~1.2us of oproj filler covers the 2-chunk exp pipeline
                # fill at each band start
                fill.drain_until(fill.pe + 1200)
                emit_band2(qkts[3], pairs[3], g)
            # remaining o_proj emitted LAST as PE filler-of-last-resort
            fill.drain_all()
            for _ in oproj_steps(3, pairs[3], last=True):
                pass

    nc.compile()
    return nc


def _get_program():
    if "nc" not in _CACHE:
        _CACHE["nc"] = _build_program()
    return _CACHE["nc"]


def _host_inputs(x, Wqkv, bqkv, Wo, bo):
    """Per-core input maps (host-side layout prep: cast/slice/fold)."""
    xT = np.ascontiguousarray(x.transpose(0, 2, 1)).astype(np.float16)

    wo16 = Wo.astype(np.float16)

    # fold v-bias through attention (softmax rows sum to 1) into o_proj bias:
    # boeff_h = bo + bv_h @ sum_w Wo[w*64+d, :]
    wsum = Wo.reshape(16, 64, E).sum(axis=0)      # [64, E] float32

    k_idx = np.arange(128)[:, None]
    q_idx = np.arange(128)[None, :]
    tri = (k_idx <= q_idx).astype(np.float16)
    trimask2 = np.concatenate([tri, tri], axis=1)  # [128, 256]
    eye = np.eye(128, dtype=np.float16)

    in_maps = []
    for c in range(NCORES):
        h0, h1 = HPC * c, HPC * c + 1
        qcols = list(range(h0 * 3 * D, h0 * 3 * D + 64)) + list(
            range(h1 * 3 * D, h1 * 3 * D + 64)
        )
        kcols = [cc + 64 for cc in qcols]
        vcols = [cc + 128 for cc in qcols]
        bqk_arr = np.stack(
            [bqkv[qcols].astype(np.float32), bqkv[kcols].astype(np.float32)], axis=1
        )  # [128, 2]
        boeff = np.zeros((128, 2 * E), np.float32)
        for i, h in enumerate((h0, h1)):
            bv = bqkv[h * 3 * D + 128 : h * 3 * D + 192].astype(np.float32)
            boeff[:, i * E : (i + 1) * E] = (bo.astype(np.float32) + bv @ wsum)[None, :]
        in_maps.append(
            {
                "xT": xT,
                "wqk": np.ascontiguousarray(Wqkv[:, qcols + kcols]).astype(np.float16),
                "wv": np.ascontiguousarray(Wqkv[:, vcols]).astype(np.float16),
                "bqk": np.ascontiguousarray(bqk_arr),
                "wo": wo16,
                "boeff": boeff,
                "trimask2": trimask2,
                "eye": eye,
            }
        )
    return in_maps


def kernel(x, mask, Wqkv, bqkv, Wo, bo, _n_cores=NCORES, _trace=False):
    """Full-input, full-output MHA. `mask` is the causal tril mask (hardcoded)."""
    from concourse.bass_utils import run_bass_kernel_spmd

    nc = _get_program()
    in_maps = _host_inputs(
        np.asarray(x), np.asarray(Wqkv), np.asarray(bqkv), np.asarray(Wo), np.asarray(bo)
    )[:_n_cores]
    res = run_bass_kernel_spmd(
        nc, in_maps, core_ids=list(range(_n_cores)), trace=_trace
    )
    out_full = np.zeros((B, S, E), np.float32)
    for c in range(_n_cores):
        o = res.results[c]["out"]  # [B, HPC, 128, E]
        for h in range(HPC):
            g = HPC * c + h
            out_full[:, g * 128 : (g + 1) * 128, :] = o[:, h]
    _CACHE["last_results"] = res
    return out_full


# revision 58
# speedup vs baseline: 1.0433x; 1.0057x over previous
"""Trainium2 Bass kernel for nn_MultiHeadAttention_46093589021200.

Causal MHA: B=4, S=2048, E=1024, H=16, D=64, with the reference's
"no-transpose-back" reshape (b,h,s,d)->(b,s,e) before the output projection.

Sharding: pure head-parallel, 2 heads per core, zero collectives.
Because of the reshape quirk, output rows s' in [h*128,(h+1)*128) depend only
on head h, so each core produces two independent 128-row output bands per
batch.

v3 design notes (vs v2; 241624ns baseline):
  - PV restructured to put q on PSUM partitions: att2[q,65] accumulates
    exT_chunk[k,q] @ [v_h|ones][k,65] over k-chunks. Cost model charges
    N(=65) per matmul instead of the q-band width (<=512), cutting PV from
    17408 to 8840 cycles per head-batch. The single ones column accumulates
    the softmax row-sum (replacing v2's 64-wide ones block).
  - softmax normalization becomes a per-partition scalar op: reciprocal of
    att2[:,64] then tensor_scalar_mul -> norm [q, (h,d)] f16 in SBUF.
  - o_proj needs (w,d)-on-partitions, so one PE transpose (via identity
    matmul) per 128-q-chunk converts norm [q,128] -> [128,q] f16 in a
    bitcast region of the att2 PSUM tile (banks are exactly full: 2x sc
    [128,1024] + att2 [128,1024] + 2x acc [128,512] = 8 banks). DVE then
    scatters into the o_proj "pair" layout (partition (w%2)*64+d, col
    (w//2)*128+u, with q = u*16+w).
  - with PV halved, attention bands are locally ACT(exp)-bound, and the PE
    executes in order -- so next-batch projection chains and o_proj(b-1) are
    interleaved at CHUNK granularity via a filler-generator queue (v2's
    en-bloc emission after bands would stall behind exp sems). PV for chunk
    kj is emitted after scores(kj+1) so the exp(kj) sem has settled.
  - per-core PE cycles: qk 131072 + v 65536 + scores 139264 + pv 70720 +
    transpose 8192 + o_proj 65536 = 480320 (~200.2us at 2.4GHz).

HW rules learned by probing (CoreSim accepts all of these, HW does not):
  - matmuls from DIFFERENT PE row groups must not write the same PSUM bank
    (same-row-group region-sharing of a bank is fine).
  - column-positioned matmuls (tile_position=(0,32j), PSUM output at a
    partition offset) mis-execute. Matmul lhsT/rhs share their SBUF base
    partition. DVE ops MAY write partition-shifted outputs.
"""

import sys

if "/opt/trn_rl_repo" not in sys.path:
    sys.path.insert(0, "/opt/trn_rl_repo")

import numpy as np

B, S, E, H = 4, 2048, 1024, 16
D = E // H          # 64
NCORES = 8
HPC = H // NCORES   # heads per core = 2
SCALE = 1.0 / float(np.sqrt(D))
NC = S // 128       # 16 key chunks

_CACHE = {}

PE_NS = 1.0 / 2.4   # ns per streamed output column at full pstate
ACT_NS = 1.0 / 1.2  # ns per lane-element on the activation engine


class Filler:
    """Two-priority queue of emission generators, drained in cost-budgeted
    slices.

    Generators emit instructions between yields; each yield value is the
    PE-cost (ns) of what was just emitted. Band emission pulls from this
    queue to keep the in-order PE stream fed through ACT-bound stretches.
    proj generators (hard emission deadline: before the next batch's bands)
    drain first; oproj generators are deliberately held back so the final
    batch -- which has no next-batch projection -- still has PE filler.
    """

    def __init__(self):
        self.projq = []
        self.oprojq = []
        self.pe = 0.0    # cumulative PE ns emitted (bands + fillers)
        self.act = 0.0   # cumulative ACT ns emitted

    def add(self, gen, proj=True):
        (self.projq if proj else self.oprojq).append(gen)

    def drain_until(self, target):
        """Pull filler until cumulative emitted PE work reaches target."""
        while self.pe < target:
            q = self.projq if self.projq else self.oprojq
            if not q:
                return
            try:
                self.pe += max(next(q[0]), 1.0)
            except StopIteration:
                q.pop(0)

    def drain_min(self, ns):
        """Pull ~ns of filler, but never run the emitted-PE clock more than
        ~1.2us ahead of the ACT clock (over-pulling here exhausts the oproj
        reserve before the last batch needs it)."""
        self.drain_until(min(self.pe + ns, self.act + 1200))

    def drain_gen(self, gen):
        """Force-finish one generator (emission-order deadline)."""
        if gen in self.projq:
            self.projq.remove(gen)
        for c in gen:
            self.pe += max(c, 1.0)

    def drain_all(self):
        for q in (self.projq, self.oprojq):
            while q:
                g = q.pop(0)
                for c in g:
                    self.pe += max(c, 1.0)


def _build_program():
    import concourse.bass as bass  # noqa: F401
    import concourse.tile as tile
    from concourse import bacc, mybir

    f16 = mybir.dt.float16
    f32 = mybir.dt.float32
    Exp = mybir.ActivationFunctionType.Exp

    nc = bacc.Bacc("TRN2", target_bir_lowering=False, debug=False)

    xT = nc.dram_tensor("xT", [B, E, S], f16, kind="ExternalInput")
    wqk = nc.dram_tensor("wqk", [E, 256], f16, kind="ExternalInput")
    wv = nc.dram_tensor("wv", [E, 128], f16, kind="ExternalInput")
    bqk = nc.dram_tensor("bqk", [128, 2], f32, kind="ExternalInput")
    wo = nc.dram_tensor("wo", [E, E], f16, kind="ExternalInput")
    boeff = nc.dram_tensor("boeff", [128, 2 * E], f32, kind="ExternalInput")
    trimask2 = nc.dram_tensor("trimask2", [128, 256], f16, kind="ExternalInput")
    eye = nc.dram_tensor("eye", [128, 128], f16, kind="ExternalInput")
    out = nc.dram_tensor("out", [B, HPC, 128, E], f32, kind="ExternalOutput")

    with tile.TileContext(nc) as tc:
        with (
            tc.tile_pool(name="const", bufs=1) as cp,
            tc.tile_pool(name="sb", bufs=2) as sb,
            tc.tile_pool(name="sb3", bufs=3) as sb3,
            tc.tile_pool(name="ps", bufs=2, space="PSUM") as ps,
        ):
            # ---- constants resident in SBUF for the whole kernel ----
            # ec0 slice first (364ns) so the first matmul unblocks early;
            # the remainder queues behind the first xt piece
            wqk_sb = cp.tile([128, 8 * 256], f16)     # [p, ec*256 + col]
            wqk_dram = wqk.ap().rearrange("(ec p) c -> p ec c", p=128)
            nc.sync.dma_start(wqk_sb[:, 0:256], wqk_dram[:, 0])
            # wv/bqk/trimask DMAs are deferred until after xt(0) (see below)
            wv_sb = cp.tile([128, 8 * 128], f16)      # [p, ec*128 + vcol]
            bqk_sb = cp.tile([128, 2], f32)
            trimask_sb = cp.tile([128, 256], f16)
            eye_sb = cp.tile([128, 128], f16)
            # o_proj constants allocated here, DMA'd after the prologue so
            # they don't delay xt(0) on the shared DMA engines
            wo_sb = cp.tile([128, 8 * E], f16)        # [p, j*1024 + c]
            boeff_sb = cp.tile([128, 2 * E], f32)     # [p, h*1024 + c], bcast rows

            # persistent double-buffered v tiles: per s-chunk c the 130-col
            # group [v_h0(64) | one | v_h1(64) | one]; the single ones column
            # accumulates the softmax row-sum during PV.
            # v2-style 256-stride v_aug tiles [v_h0|ones(64)|v_h1|ones(64)]:
            # 256-aligned moving-operand offsets (the earlier 130-stride
            # layout put PV rhs at odd 130-byte offsets, which mis-executes
            # on HW), ones col 64 doubles as the v3-band rowsum column and
            # the 64-wide block serves the v2-style last-batch band.
            v3t = [cp.tile([128, NC * 256], f16, name=f"v3_{i}") for i in range(2)]
            for t in v3t:
                tv = t.rearrange("p (c h z) -> p c h z", c=NC, h=2)
                nc.gpsimd.memset(tv[:, :, :, 64:128], 1.0)

            def emit_xt_dma(b, fine=False, wqk_rest=None):
                # 8 per-ec tiles so projection chains start as each DMA lands.
                # fine=True (first batch): split per (n-block, ec) so the
                # prologue chains complete incrementally from ~3us.
                xts = [
                    sb.tile([128, S], f16, tag=f"xt{ec}", name=f"xt{ec}", bufs=3)
                    for ec in range(8)
                ]
                xt_dram = xT.ap()[b].rearrange("(ec p) s -> p ec s", p=128)
                if fine:
                    # halves: 728ns transfer > 625ns HWDGE overhead, so the
                    # piece stream stays transfer-bound. The very first piece
                    # is a 512-col quarter: exactly what the first projection
                    # matmul needs, so it unblocks ~360ns earlier.
                    for n in range(2):
                        for ec in range(8):
                            if ec == 0 and n == 0:
                                nc.sync.dma_start(
                                    xts[0][:, 0:512], xt_dram[:, 0, 0:512]
                                )
                                nc.sync.dma_start(
                                    xts[0][:, 512:1024], xt_dram[:, 0, 512:1024]
                                )
                            else:
                                nc.sync.dma_start(
                                    xts[ec][:, n * 1024 : (n + 1) * 1024],
                                    xt_dram[:, ec, n * 1024 : (n + 1) * 1024],
                                )
                            if wqk_rest is not None and ec == 0 and n == 0:
                                wsb, wdr = wqk_rest
                                nc.sync.dma_start(
                                    wsb.rearrange("p (ec c) -> p ec c", ec=8)[
                                        :, 1:8
                                    ],
                                    wdr[:, 1:8],
                                )
                else:
                    for ec in range(8):
                        nc.sync.dma_start(xts[ec], xt_dram[:, ec])
                return xts

            def emit_v_group(b, xts, sc4, tag="acc", bufs=2):
                # v in natural [s, d] layout, 4 s-chunks per PSUM bank
                v3_sb = v3t[b % 2]
                vq = ps.tile([128, 512], f32, tag=tag, name="vq", bufs=bufs)
                for sub in range(4):
                    c = sc4 * 4 + sub
                    for ec in range(8):
                        nc.tensor.matmul(
                            vq[:, sub * 128 : (sub + 1) * 128],
                            xts[ec][:, c * 128 : (c + 1) * 128],
                            wv_sb[:, ec * 128 : (ec + 1) * 128],
                            start=(ec == 0),
                            stop=(ec == 7),
                        )
                # copy into v3 chunks (skips the ones columns)
                nc.vector.tensor_copy(
                    v3_sb.rearrange("p (c h z) -> p c h z", c=NC, h=2)[
                        :, sc4 * 4 : sc4 * 4 + 4, :, 0:64
                    ],
                    vq.rearrange("p (c h dd) -> p c h dd", c=4, h=2),
                )

            def emit_proj0(b, xts):
                # prologue-only en-bloc projection (DMA-paced anyway):
                # q/k chains then v groups, borrowing the still-idle PSUM
                # tags so several chains progress as the fine pieces land.
                # only n=0,1 here: the n=2,3 chains are deferred into the
                # filler queue as READY work for bands(0), whose natural
                # filler (proj(1)) is blocked on the xt(1) DMA until ~26us.
                tags = [("att2", 1), ("sc", 2), ("acc", 2)]
                qkT_sb = sb.tile([128, 2 * S], f16, tag="qkT", name="qkT")
                for n in range(2):
                    for m in range(2):
                        tg, bf = tags[(n * 2 + m) % 3]
                        pq = ps.tile([128, 512], f32, tag=tg, name="pq", bufs=bf)
                        for ec in range(8):
                            nc.tensor.matmul(
                                pq,
                                wqk_sb[:, ec * 256 + m * 128 : ec * 256 + (m + 1) * 128],
                                xts[ec][:, n * 512 : (n + 1) * 512],
                                start=(ec == 0),
                                stop=(ec == 7),
                            )
                        nc.vector.tensor_scalar_add(
                            qkT_sb[:, m * S + n * 512 : m * S + (n + 1) * 512],
                            pq,
                            bqk_sb[:, m : m + 1],
                        )
                vtags = [("sc", 2), ("att2", 1)]
                for sc4 in range(2):
                    tg, bf = vtags[sc4]
                    emit_v_group(b, xts, sc4, tag=tg, bufs=bf)
                return qkT_sb

            def proj_steps(b, xts, qkT_sb, ns=range(4)):
                # filler generator: q/k chain pair then v group, per n-block
                # (matches the order bands(b) consume them: band g needs
                # qk n<=g, k all ... emitted ascending; v groups 0..g).
                for n in ns:
                    for m in range(2):
                        pq = ps.tile([128, 512], f32, tag="acc", name="pq")
                        for ec in range(8):
                            nc.tensor.matmul(
                                pq,
                                wqk_sb[:, ec * 256 + m * 128 : ec * 256 + (m + 1) * 128],
                                xts[ec][:, n * 512 : (n + 1) * 512],
                                start=(ec == 0),
                                stop=(ec == 7),
                            )
                            yield 512 * PE_NS
                        nc.vector.tensor_scalar_add(
                            qkT_sb[:, m * S + n * 512 : m * S + (n + 1) * 512],
                            pq,
                            bqk_sb[:, m : m + 1],
                        )
                        yield 0.0
                    v_sb = v3t[b % 2]
                    vq = ps.tile([128, 512], f32, tag="acc", name="vq")
                    for sub in range(4):
                        c = n * 4 + sub
                        for ec in range(8):
                            nc.tensor.matmul(
                                vq[:, sub * 128 : (sub + 1) * 128],
                                xts[ec][:, c * 128 : (c + 1) * 128],
                                wv_sb[:, ec * 128 : (ec + 1) * 128],
                                start=(ec == 0),
                                stop=(ec == 7),
                            )
                            yield 128 * PE_NS
                    nc.vector.tensor_copy(
                        v_sb.rearrange("p (c h z) -> p c h z", c=NC, h=2)[
                            :, n * 4 : n * 4 + 4, :, 0:64
                        ],
                        vq.rearrange("p (c h dd) -> p c h dd", c=4, h=2),
                    )
                    yield 0.0

            def oproj_steps(b, pair, last=False):
                # o_proj: po[u, c] = sum_j pair[h][:, j*128:+128]^T wo_j
                # bias added on DVE during the PSUM->SBUF copy
                for h in range(2):
                    out_sb = sb.tile([128, E], f32, tag="osb", name="osb")
                    for n2 in range(2):
                        po = ps.tile([128, 512], f32, tag="acc", name="po")
                        for j in range(8):
                            nc.tensor.matmul(
                                po,
                                pair[h][:, j * 128 : (j + 1) * 128],
                                wo_sb[:, j * E + n2 * 512 : j * E + (n2 + 1) * 512],
                                start=(j == 0),
                                stop=(j == 7),
                            )
                            yield 512 * PE_NS
                        # all stores ride the SP ring: ACT ring config would
                        # stall the exp pacer; Pool ring SWDGE generation
                        # (~1us/store) would delay the diag mask-muls that
                        # gate PV
                        ring = nc.sync
                        if last and h == 1:
                            # final two chains: 256-col DVE+DMA pieces on
                            # alternating rings pipeline the kernel-exit tail
                            for z in range(2):
                                cl = n2 * 512 + z * 256
                                nc.vector.tensor_add(
                                    out_sb[:, cl : cl + 256],
                                    po[:, z * 256 : (z + 1) * 256],
                                    boeff_sb[:, h * E + cl : h * E + cl + 256],
                                )
                                rg = nc.sync if z == 0 else nc.scalar
                                rg.dma_start(
                                    out.ap()[b, h, :, cl : cl + 256],
                                    out_sb[:, cl : cl + 256],
                                )
                        else:
                            nc.vector.tensor_add(
                                out_sb[:, n2 * 512 : (n2 + 1) * 512],
                                po,
                                boeff_sb[:, h * E + n2 * 512 : h * E + (n2 + 1) * 512],
                            )
                            ring.dma_start(
                                out.ap()[b, h, :, n2 * 512 : (n2 + 1) * 512],
                                out_sb[:, n2 * 512 : (n2 + 1) * 512],
                            )
                        yield 0.0

            # ---- attention over 4 bands of 512 queries ----
            def emit_band2(qkT_sb, pair, g):
                # v2-style band for the LAST batch: PE-dominated per chunk
                # (no filler needed), 64-wide-ones rowsum replication, DVE
                # normalization scatter at band end. atts = the two banks of
                # one att2-tag tile.
                att2_t = ps.tile([128, 1024], f32, tag="att2", name="att2",
                                 bufs=1)
                atts = [att2_t[:, 0:512], att2_t[:, 512:1024]]
                nkj = 4 * g + 4

                def emit_pv2(kj, qo, ex):
                    for h in range(2):
                        nc.tensor.matmul(
                            atts[h][:, qo:512],
                            v3t[1][:, kj * 256 + h * 128 : kj * 256 + (h + 1) * 128],
                            ex[:, h * 512 + qo : (h + 1) * 512],
                            start=(kj == 0),
                            stop=(kj == nkj - 1),
                        )

                pend = []
                for kj in range(nkj):
                    qo = 128 * max(0, kj - 4 * g)
                    scp = ps.tile([128, 1024], f32, tag="sc", name="scp", bufs=2)
                    ex = sb3.tile([128, 1024], f16, tag="ex", name="ex", bufs=9)
                    for h in range(2):
                        nc.tensor.matmul(
                            scp[:, h * 512 + qo : (h + 1) * 512],
                            qkT_sb[h * 64 : (h + 1) * 64,
                                   S + kj * 128 : S + (kj + 1) * 128],
                            qkT_sb[h * 64 : (h + 1) * 64,
                                   g * 512 + qo : (g + 1) * 512],
                            start=True,
                            stop=True,
                            tile_position=(h * 64, 0),
                        )
                    nc.scalar.activation(
                        ex.rearrange("p (h q) -> p h q", h=2)[:, :, qo:512],
                        scp.rearrange("p (h q) -> p h q", h=2)[:, :, qo:512],
                        Exp,
                        scale=SCALE,
                    )
                    if kj >= 4 * g:  # diagonal chunk: zero q < k
                        nc.vector.tensor_mul(
                            ex.rearrange("p (h q) -> p h q", h=2)[
                                :, :, qo : qo + 128
                            ],
                            ex.rearrange("p (h q) -> p h q", h=2)[
                                :, :, qo : qo + 128
                            ],
                            trimask_sb.rearrange("p (h q) -> p h q", h=2),
                        )
                    # off-diagonal PV trails scores by one chunk (exp sem
                    # settled); diag immediate so the band end is not delayed
                    pend.append((kj, qo, ex))
                    if kj >= 4 * g - 1:
                        while pend:
                            emit_pv2(*pend.pop(0))
                    elif len(pend) > 1:
                        emit_pv2(*pend.pop(0))
                # normalize + scatter into o_proj pair layout
                for h in range(2):
                    rb = sb.tile([64, 512], f32, tag="rb", name="rb")
                    nc.vector.reciprocal(rb, atts[h][64:128, :])
                    attv = atts[h].rearrange(
                        "p (u2 w2 pr) -> p u2 w2 pr", u2=32, w2=8
                    )
                    rbv = rb.rearrange(
                        "p (u2 w2 pr) -> p u2 w2 pr", u2=32, w2=8
                    )
                    pav = pair[h].rearrange("p (j u) -> p u j", j=8)
                    for par in range(2):
                        nc.vector.tensor_mul(
                            pav[par * 64 : (par + 1) * 64,
                                g * 32 : (g + 1) * 32, :],
                            attv[0:64, :, :, par : par + 1],
                            rbv[0:64, :, :, par : par + 1],
                        )

            def emit_band(b, qkT_sb, pair, g, fill, fine_tail=False):
                # PSUM deps are tracked at BANK granularity: any read of an
                # accumulating bank serializes later matmul writes to it.
                # So: bank0 = h0 data [0:256) + 4 f16 transpose slots
                # [256:512); bank1 = h1 data [512:768) + all rowsums
                # [768:776). ALL normalization reads happen at band end,
                # after both banks' accumulations have stopped.
                v3_sb = v3t[b % 2]
                att2 = ps.tile([128, 1024], f32, tag="att2", name="att2", bufs=1)
                trv = att2[:, 256:512].bitcast(f16)   # [128, 512] f16

                def emit_pv(kj, lo, ex):
                    cost = 0.0
                    for h in range(2):
                        for qc in range(lo, 4):
                            exs = ex[:, h * 512 + qc * 128 : h * 512 + (qc + 1) * 128]
                            nc.tensor.matmul(
                                att2[:, h * 512 + qc * 64 : h * 512 + (qc + 1) * 64],
                                exs,
                                v3_sb[:, kj * 256 + h * 128 : kj * 256 + h * 128 + 64],
                                start=(kj == 0 and h == 0 and qc == 0),
                                stop=(kj == 4 * g + qc),
                                skip_group_check=True,
                            )
                            # softmax row-sum rides as a 1-column matmul of
                            # the ones column (same stationary ex chunk)
                            nc.tensor.matmul(
                                att2[:, 768 + h * 4 + qc : 768 + h * 4 + qc + 1],
                                exs,
                                v3_sb[:, kj * 256 + h * 128 + 64 : kj * 256 + h * 128 + 65],
                                start=(kj == 0 and h == 0 and qc == 0),
                                stop=(kj == 4 * g + qc),
                                skip_group_check=True,
                            )
                            cost += 65 * PE_NS
                    return cost

                nkj = 4 * g + 4
                pend = []
                for kj in range(nkj):
                    qo = 128 * max(0, kj - 4 * g)
                    lo = max(0, kj - 4 * g)
                    # slot order: PV/filler first, scores LAST — this gives
                    # the scp WAR (vs the exp two chunks back) extra slack
                    if len(pend) > 6:
                        fill.pe += emit_pv(*pend.pop(0))
                    fill.drain_until(fill.act)
                    scp = ps.tile([128, 1024], f32, tag="sc", name="scp", bufs=2)
                    ex = sb3.tile([128, 1024], f16, tag="ex", name="ex", bufs=9)
                    fill.pe += 2 * (512 - qo) * PE_NS
                    for h in range(2):
                        nc.tensor.matmul(
                            scp[:, h * 512 + qo : (h + 1) * 512],
                            qkT_sb[h * 64 : (h + 1) * 64,
                                   S + kj * 128 : S + (kj + 1) * 128],
                            qkT_sb[h * 64 : (h + 1) * 64,
                                   g * 512 + qo : (g + 1) * 512],
                            start=True,
                            stop=True,
                            tile_position=(h * 64, 0),
                        )
                    nc.scalar.activation(
                        ex.rearrange("p (h q) -> p h q", h=2)[:, :, qo:512],
                        scp.rearrange("p (h q) -> p h q", h=2)[:, :, qo:512],
                        Exp,
                        scale=SCALE,
                    )
                    fill.act += 2 * (512 - qo) * ACT_NS + 190
                    if kj >= 4 * g:  # diagonal chunk: zero q < k
                        # on the idle Pool engine: DVE is near-saturated
                        # during bands with norm/scatter/finalize work
                        nc.gpsimd.tensor_mul(
                            ex.rearrange("p (h q) -> p h q", h=2)[
                                :, :, qo : qo + 128
                            ],
                            ex.rearrange("p (h q) -> p h q", h=2)[
                                :, :, qo : qo + 128
                            ],
                            trimask_sb.rearrange("p (h q) -> p h q", h=2),
                        )
                    pend.append((kj, lo, ex))
                while pend:
                    fill.pe += emit_pv(*pend.pop(0))
                    fill.drain_min(400)
                # ---- band-end normalization block ----
                # one recip (all 8 rowsums), 8 muls, 4 transposes, 4
                # batched scatters; bank transitions are single-direction
                # (reads of data banks, then writes to bank0 tr slots)
                rcp = sb.tile([128, 8], f32, tag="rcp", name="rcp", bufs=2)
                nc.vector.reciprocal(rcp, att2[:, 768:776])
                nrms = []
                for qc in range(4):
                    nrm = sb.tile([128, 128], f16, tag="nrm", name="nrm",
                                  bufs=4)
                    for h in range(2):
                        nc.vector.tensor_scalar_mul(
                            nrm[:, h * 64 : (h + 1) * 64],
                            att2[:, h * 512 + qc * 64 : h * 512 + (qc + 1) * 64],
                            rcp[:, h * 4 + qc : h * 4 + qc + 1],
                        )
                    nrms.append(nrm)
                    fill.drain_min(350)
                fill.drain_min(600)
                for qc in range(4):
                    # start=False: land on the region zeroed by the band's
                    # PV start (HW zeroes the whole bank on start=True)
                    nc.tensor.matmul(
                        trv[:, qc * 128 : (qc + 1) * 128], nrms[qc], eye_sb,
                        is_transpose=True, start=False, stop=True,
                        skip_group_check=True,
                    )
                    fill.pe += 128 * PE_NS
                fill.drain_min(500)
                trb = trv.rearrange(
                    "p (qc ul j pr) -> p pr j qc ul", qc=4, ul=8, j=8
                )
                for h in range(2):
                    pav = pair[h].rearrange(
                        "p (j qcg ul) -> p j qcg ul", j=8, qcg=16
                    )
                    for par in range(2):
                        nc.vector.tensor_copy(
                            pav[par * 64 : (par + 1) * 64, :,
                                g * 4 : g * 4 + 4, :],
                            trb[h * 64 : (h + 1) * 64, par],
                        )
                # guard reads: the next band's start=True PVs zero their
                # whole bank (invisible to the range tracker); these trailing
                # DVE reads of cols 0 and 768 give those PVs a WAR dep that
                # orders the zeroing after this band's scatters/muls
                grd = sb.tile([128, 2], f32, tag="grd", name="grd", bufs=2)
                nc.vector.tensor_copy(grd[:, 0:1], att2[:, 0:1])
                nc.vector.tensor_copy(grd[:, 1:2], att2[:, 768:769])
                fill.drain_min(400)

            # ================= prologue ==================================
            # SP DMA queue order is the schedule —
            # wqk | xt(0) fine | wv | xt(1) | wo+boeff | xt(2) ...
            # small consts ride the Pool/SWDGE path, off the HWDGE queue
            nc.gpsimd.dma_start(bqk_sb, bqk.ap())
            nc.gpsimd.dma_start(trimask_sb, trimask2.ap())
            nc.gpsimd.dma_start(eye_sb, eye.ap())
            xts0 = emit_xt_dma(0, fine=True, wqk_rest=(wqk_sb, wqk_dram))
            xtss = {}
            nc.sync.dma_start(
                wv_sb.rearrange("p (ec c) -> p ec c", ec=8),
                wv.ap().rearrange("(ec p) c -> p ec c", p=128),
            )
            qkts = {0: emit_proj0(0, xts0)}
            xtss[1] = emit_xt_dma(1)
            nc.sync.dma_start(
                wo_sb.rearrange("p (j c) -> p j c", j=8),
                wo.ap().rearrange("(j p) c -> p j c", p=128),
            )
            nc.sync.dma_start(boeff_sb, boeff.ap())

            fill = Filler()
            # n=2,3 chains of proj(b) are deferred INTO bands(b): band g
            # only reads qkT/v from chains n<=g. proj(3) fully drains during
            # bands(2) because the v2-style bands(3) consume no filler.
            defer = proj_steps(0, xts0, qkts[0], ns=(2, 3))
            pairs = {}
            for b in range(3):
                pairs[b] = [
                    sb.tile([128, 8 * 128], f16, tag=f"pair{h}", name=f"pair{h}",
                            bufs=4)
                    for h in range(2)
                ]
                if defer is not None:
                    fill.add(defer)
                qkts[b + 1] = sb.tile([128, 2 * S], f16, tag="qkT", name="qkT")
                xts_n = xtss.pop(b + 1)
                if b < 2:
                    proj_gen = proj_steps(b + 1, xts_n, qkts[b + 1], ns=(0, 1))
                    defer_next = proj_steps(b + 1, xts_n, qkts[b + 1],
                                            ns=(2, 3))
                else:
                    proj_gen = proj_steps(3, xts_n, qkts[3], ns=range(4))
                    defer_next = None
                fill.add(proj_gen)
                for g in range(4):
                    if g == 1 and defer is not None:
                        # deadline: band 2 reads qkT/v from n=2; one band
                        # early so the v-copy DVE latency is hidden
                        fill.drain_gen(defer)
                    emit_band(b, qkts[b], pairs[b], g, fill)
                # hard deadline: proj(b+1) emitted before bands(b+1)
                fill.drain_gen(proj_gen)
                fill.add(oproj_steps(b, pairs[b]), proj=False)
                defer = defer_next
                del qkts[b]
                if b + 2 < B:
                    xtss[b + 2] = emit_xt_dma(b + 2)
            # last batch: v2-style self-filling bands (PE-dominated chunks)
            pairs[3] = [
                sb.tile([128, 8 * 128], f16, tag=f"pair{h}", name=f"pair{h}",
                        bufs=4)
                for h in range(2)
            ]
            for g in range(4):
                # ~1.2us of oproj filler covers the 2-chunk exp pipeline
                # fill at each band start
                fill.drain_until(fill.pe + 1200)
                emit_band2(qkts[3], pairs[3], g)
            # remaining o_proj emitted LAST as PE filler-of-last-resort
            fill.drain_all()
            for _ in oproj_steps(3, pairs[3], last=True):
                pass

    nc.compile()
    return nc


def _get_program():
    if "nc" not in _CACHE:
        _CACHE["nc"] = _build_program()
    return _CACHE["nc"]


def _host_inputs(x, Wqkv, bqkv, Wo, bo):
    """Per-core input maps (host-side layout prep: cast/slice/fold)."""
    xT = np.ascontiguousarray(x.transpose(0, 2, 1)).astype(np.float16)

    wo16 = Wo.astype(np.float16)

    # fold v-bias through attention (softmax rows sum to 1) into o_proj bias:
    # boeff_h = bo + bv_h @ sum_w Wo[w*64+d, :]
    wsum = Wo.reshape(16, 64, E).sum(axis=0)      # [64, E] float32

    k_idx = np.arange(128)[:, None]
    q_idx = np.arange(128)[None, :]
    tri = (k_idx <= q_idx).astype(np.float16)
    trimask2 = np.concatenate([tri, tri], axis=1)  # [128, 256]
    eye = np.eye(128, dtype=np.float16)

    in_maps = []
    for c in range(NCORES):
        h0, h1 = HPC * c, HPC * c + 1
        qcols = list(range(h0 * 3 * D, h0 * 3 * D + 64)) + list(
            range(h1 * 3 * D, h1 * 3 * D + 64)
        )
        kcols = [cc + 64 for cc in qcols]
        vcols = [cc + 128 for cc in qcols]
        bqk_arr = np.stack(
            [bqkv[qcols].astype(np.float32), bqkv[kcols].astype(np.float32)], axis=1
        )  # [128, 2]
        boeff = np.zeros((128, 2 * E), np.float32)
        for i, h in enumerate((h0, h1)):
            bv = bqkv[h * 3 * D + 128 : h * 3 * D + 192].astype(np.float32)
            boeff[:, i * E : (i + 1) * E] = (bo.astype(np.float32) + bv @ wsum)[None, :]
        in_maps.append(
            {
                "xT": xT,
                "wqk": np.ascontiguousarray(Wqkv[:, qcols + kcols]).astype(np.float16),
                "wv": np.ascontiguousarray(Wqkv[:, vcols]).astype(np.float16),
                "bqk": np.ascontiguousarray(bqk_arr),
                "wo": wo16,
                "boeff": boeff,
                "trimask2": trimask2,
                "eye": eye,
            }
        )
    return in_maps


def kernel(x, mask, Wqkv, bqkv, Wo, bo, _n_cores=NCORES, _trace=False):
    """Full-input, full-output MHA. `mask` is the causal tril mask (hardcoded)."""
    from concourse.bass_utils import run_bass_kernel_spmd

    nc = _get_program()
    in_maps = _host_inputs(
        np.asarray(x), np.asarray(Wqkv), np.asarray(bqkv), np.asarray(Wo), np.asarray(bo)
    )[:_n_cores]
    res = run_bass_kernel_spmd(
        nc, in_maps, core_ids=list(range(_n_cores)), trace=_trace
    )
    out_full = np.zeros((B, S, E), np.float32)
    for c in range(_n_cores):
        o = res.results[c]["out"]  # [B, HPC, 128, E]
        for h in range(HPC):
            g = HPC * c + h
            out_full[:, g * 128 : (g + 1) * 128, :] = o[:, h]
    _CACHE["last_results"] = res
    return out_full
